# revision 22
# baseline (speedup 1.0000x reference)
"""Trainium2 Bass kernel for the TGAT-style AttnModel (gnn_message_passing).

Contract: kernel(**inputs) takes FULL unsharded numpy inputs (as produced by
setup_inputs()) and returns the FULL output tuple (z, attn).

Strategy: pure data parallel over batch B=16 -> 2 batches per NeuronCore
(8 cores). Per core: 64 sources, 4096 neighbor rows. All projections run as
fp32r matmuls on the PE; attention uses a block-diagonal-Q trick for QK and a
block-diagonal-probs (BD) trick for attn@V; softmax in fp32 on DVE/ACT.
Host-side prep does the concats/transposes (pure data movement).
"""

import os
import numpy as np

import ml_dtypes

# ---- model constants (hardcoded; kernel.py must be self-contained) ----
B, NS, NGH, NBR = 16, 32, 2048, 64
FEAT = 256
DM, H, DK = 1024, 8, 128
LN_EPS = 1e-5

NCORES = 8
BPC = B // NCORES          # 2 batches per core
S = BPC * NS               # 64 sources per core
ROWS = BPC * NGH           # 4096 k-rows per core
NBLK = 8
BLK = ROWS // NBLK         # 512 rows per block
SPB = S // NBLK            # 8 sources per block
RC = BLK // 128            # 4 row-chunks of 128 per block

_CACHE = {}
LAST_EXEC_NS = None


def _patch_drain():
    """walrus in this container rejects instructions with >1 sync wait on the
    Drain ctrl struct; split the TileContext tail-drain waits into one drain
    per semaphore."""
    import concourse.tile as tile
    from concourse import mybir
    from concourse.vector_clock import ScopedClock

    if getattr(tile.TileContext, "_drain_patched", False):
        return

    def _drain_and_barrier(self, tick_clock, wait_clock):
        nc = self.nc
        drain_inst = nc.sync.drain()
        wait_clock.add_sem_waits(
            drain_inst.ins, ScopedClock({None: tick_clock.global_clock})
        )
        ri = drain_inst.ins
        waits = list(ri.sync_info.on_wait)
        ri.sync_info = mybir.SyncInfo(on_wait=waits[:1], on_update=[])
        for w in waits[1:]:
            d2 = nc.sync.drain()
            d2.ins.sync_info = mybir.SyncInfo(on_wait=[w], on_update=[])
        nc.all_engine_barrier()
        popped = nc._tile_sem_poison_stack.pop()
        assert popped is self._sem_poison
        nc.clear_and_free_semaphores(list(self.sems.allocated().values()))
        nc.all_engine_barrier()

    tile.TileContext._drain_and_barrier = _drain_and_barrier
    tile.TileContext._drain_patched = True


def _fix_multiwait(nc):
    """walrus in this container accepts very few sync commands per
    instruction (1 wait on NOP/Drain ctrl; a wait+update on engine ctrl).
    Conservatively rewrite every block so each instruction carries at most
    ONE wait: excess waits move to same-engine NoOp carriers inserted
    immediately before the instruction — identical semantics (same engine,
    same program position), so no scheduling or deadlock risk."""
    from concourse import mybir

    ctr = [0]
    for f in nc.m.functions:
        for bb in f.blocks:
            out = []
            changed = False
            for inst in bb.instructions:
                si = inst.sync_info
                if si is not None and len(si.on_wait) > 1:
                    waits = list(si.on_wait)
                    for w in waits[:-1]:
                        ctr[0] += 1
                        nop = mybir.InstNoOp(
                            name=f"I-wfix-{ctr[0]}", ins=[], outs=[]
                        )
                        nop.engine = inst.engine
                        nop.sync_info = mybir.SyncInfo(
                            on_wait=[w], on_update=[]
                        )
                        out.append(nop)
                    inst.sync_info = mybir.SyncInfo(
                        on_wait=[waits[-1]], on_update=list(si.on_update)
                    )
                    changed = True
                out.append(inst)
            if changed:
                bb.instructions = out


def _build_nc():
    import concourse.bass as bass
    import concourse.tile as tile
    import concourse.tile_utils as tile_utils
    from concourse import mybir
    from concourse.bass import ts
    from concourse.masks import make_identity

    _patch_drain()
    # stale 192KB cap; cayman has 208KB usable per partition
    tile_utils.max_sbuf_usage = 207 * 1024

    f32 = mybir.dt.float32
    f32r = mybir.dt.float32r
    bf16 = mybir.dt.bfloat16
    AX = mybir.AxisListType
    AF = mybir.ActivationFunctionType

    nc = bass.Bass(target_bir_lowering=False)

    # ---- DRAM parameters (per-core views, host-prepared) ----
    kt_d = nc.declare_dram_parameter("kt", [DM, ROWS], f32r, isOutput=False)
    qt_d = nc.declare_dram_parameter("qt", [DM, S], f32r, isOutput=False)
    qpfcb_d = nc.declare_dram_parameter("qpfcb", [S, DM], f32, isOutput=False)
    srcT_d = nc.declare_dram_parameter("srcT", [FEAT, S], f32r, isOutput=False)
    biasF_d = nc.declare_dram_parameter("biasF", [NBLK, SPB * H, BLK], f32, isOutput=False)
    m0_d = nc.declare_dram_parameter("m0", [128, 32, 64], bf16, isOutput=False)
    wqT_d = nc.declare_dram_parameter("wqT", [DM, DM], f32r, isOutput=False)
    wkT_d = nc.declare_dram_parameter("wkT", [DM, DM], f32r, isOutput=False)
    wvT_d = nc.declare_dram_parameter("wvT", [DM, DM], f32r, isOutput=False)
    fcT_d = nc.declare_dram_parameter("fcT", [DM, DM], f32r, isOutput=False)
    fc1T_d = nc.declare_dram_parameter("fc1T", [DM + FEAT, FEAT], f32r, isOutput=False)
    fc2T_d = nc.declare_dram_parameter("fc2T", [FEAT, FEAT], f32r, isOutput=False)
    fc1b_d = nc.declare_dram_parameter("fc1b", [S, FEAT], f32, isOutput=False)
    fc2b_d = nc.declare_dram_parameter("fc2b", [S, FEAT], f32, isOutput=False)
    z_d = nc.declare_dram_parameter("z", [S, FEAT], f32, isOutput=True)
    attn_d = nc.declare_dram_parameter("attn", [S * H, NBR], f32, isOutput=True)

    with tile.TileContext(nc) as tc:
        with (
            tc.tile_pool(name="const", bufs=1) as const,
            tc.tile_pool(name="kt", bufs=2) as ktp_pool,
            tc.tile_pool(name="kproj", bufs=1) as kproj_pool,
            tc.tile_pool(name="v", bufs=1) as v_pool,
            tc.tile_pool(name="bdq", bufs=2) as bdq_pool,
            tc.tile_pool(name="sm", bufs=2) as sm_pool,
            tc.tile_pool(name="bf", bufs=2) as bf_pool,
            tc.tile_pool(name="st", bufs=8) as st_pool,
            tc.tile_pool(name="a", bufs=2) as a_pool,
            tc.tile_pool(name="bd", bufs=2) as bd_pool,
            tc.tile_pool(name="tail", bufs=1) as tail,
            tc.tile_pool(name="kps", bufs=2, space="PSUM") as kps_pool,
            tc.tile_pool(name="vps", bufs=2, space="PSUM") as vps_pool,
            tc.tile_pool(name="qkps", bufs=1, space="PSUM") as qkps_pool,
            tc.tile_pool(name="avps", bufs=1, space="PSUM") as avps_pool,
            tc.tile_pool(name="tpps", bufs=2, space="PSUM") as tpps_pool,
        ):
            # ---- constants / weights ----
            wk_t = const.tile([128, 8, DM], f32r)
            wv_t = const.tile([128, 8, DM], f32r)
            fc_t = const.tile([128, 8, DM], f32r)  # holds wqT first, fcT later
            fc1_t = const.tile([128, 10, FEAT], f32r)
            fc2_t = const.tile([128, 2, FEAT], f32r)
            m0_t = const.tile([128, 32, 64], bf16)
            qpfcb_t = const.tile([S, DM], f32)
            fc1b_t = const.tile([S, FEAT], f32)
            fc2b_t = const.tile([S, FEAT], f32)
            xcatT_t = const.tile([128, 10, S], f32r)
            qproj_t = const.tile([128, 8, S], f32)
            ident_t = const.tile([128, 128], f32)
            eps_t = const.tile([S, 1], f32)
            zero64_t = const.tile([128, SPB * H], f32)

            for i in range(8):
                nc.sync.dma_start(out=wk_t[:, i, :], in_=wkT_d[ts(i, 128), :])
                nc.sync.dma_start(out=wv_t[:, i, :], in_=wvT_d[ts(i, 128), :])
                nc.sync.dma_start(out=fc_t[:, i, :], in_=wqT_d[ts(i, 128), :])
            for i in range(10):
                nc.sync.dma_start(out=fc1_t[:, i, :], in_=fc1T_d[ts(i, 128), :])
            for i in range(2):
                nc.sync.dma_start(out=fc2_t[:, i, :], in_=fc2T_d[ts(i, 128), :])
                nc.sync.dma_start(out=xcatT_t[:, 8 + i, :], in_=srcT_d[ts(i, 128), :])
            nc.sync.dma_start(out=m0_t[:], in_=m0_d[:])
            nc.sync.dma_start(out=qpfcb_t[:], in_=qpfcb_d[:])
            nc.sync.dma_start(out=fc1b_t[:], in_=fc1b_d[:])
            nc.sync.dma_start(out=fc2b_t[:], in_=fc2b_d[:])
            make_identity(nc, ident_t)
            nc.vector.memset(eps_t, LN_EPS)
            nc.vector.memset(zero64_t, 0.0)

            # ---- Q projection: qproj[(h,d) chunk h][d, s] ----
            # fc_t currently holds wqT; it is re-loaded with fcT below.
            qt_t = const.tile([128, 8, S], f32r)
            for i in range(8):
                nc.sync.dma_start(out=qt_t[:, i, :], in_=qt_d[ts(i, 128), :])
            for o in range(8):
                qps = kps_pool.tile([128, 512], f32, tag="mm")
                for i in range(8):
                    nc.tensor.matmul(
                        qps[:, 0:S],
                        fc_t[:, i, ts(o, 128)],
                        qt_t[:, i, :],
                        start=(i == 0),
                        stop=(i == 7),
                    )
                nc.vector.tensor_copy(qproj_t[:, o, :], qps[:, 0:S])
            # now overwrite with fcT for the tail (WAR handled by Tile)
            for i in range(8):
                nc.sync.dma_start(out=fc_t[:, i, :], in_=fcT_d[ts(i, 128), :])

            # ---- persistent AV accumulator: [d, h, s] (one PSUM bank) ----
            av_ps = avps_pool.tile([128, H, S], f32)

            for b in range(NBLK):
                # load k^T block
                kt_t = ktp_pool.tile([128, 8, BLK], f32r)
                for i in range(8):
                    nc.sync.dma_start(
                        out=kt_t[:, i, :], in_=kt_d[ts(i, 128), ts(b, BLK)]
                    )
                biasF_t = bf_pool.tile([SPB * H, BLK], f32)
                nc.sync.dma_start(out=biasF_t, in_=biasF_d[b])

                # K-projection (transposed out): ktp[(h,d) chunk][d, rows]
                ktproj_t = kproj_pool.tile([128, 8, BLK], f32r)
                for o in range(8):
                    ps = kps_pool.tile([128, 512], f32, tag="mm")
                    for i in range(8):
                        nc.tensor.matmul(
                            ps,
                            wk_t[:, i, ts(o, 128)],
                            kt_t[:, i, :],
                            start=(i == 0),
                            stop=(i == 7),
                        )
                    nc.vector.tensor_copy(ktproj_t[:, o, :], ps)

                # V-projection (normal out, bf16): v[rowchunk][row, (h,d)]
                v_t = v_pool.tile([128, RC, DM], bf16)
                for rc in range(RC):
                    for half in range(2):
                        ps = vps_pool.tile([128, 512], f32, tag="vmm")
                        for i in range(8):
                            nc.tensor.matmul(
                                ps,
                                kt_t[:, i, ts(rc, 128)],
                                wv_t[:, i, ts(half, 512)],
                                start=(i == 0),
                                stop=(i == 7),
                            )
                        nc.vector.tensor_copy(v_t[:, rc, ts(half, 512)], ps)

                # QK via block-diagonal Q: out[(s,h), rows]
                qk_ps = qkps_pool.tile([SPB * H, BLK], f32)
                for h in range(H):
                    bdq = bdq_pool.tile([128, SPB * H], f32r)
                    bdq_r = bdq.rearrange("p (s h) -> p s h", h=H)
                    nc.vector.tensor_copy(bdq, zero64_t)
                    nc.vector.tensor_copy(bdq_r[:, :, h], qproj_t[:, h, ts(b, SPB)])
                    nc.tensor.matmul(
                        qk_ps,
                        bdq,
                        ktproj_t[:, h, :],
                        start=(h == 0),
                        stop=(h == 7),
                    )

                # mask/bias over the FULL row (off-diagonal cols get -1e10,
                # so softmax over 512 cols == per-source softmax over 64)
                smf = sm_pool.tile([SPB * H, BLK], f32, tag="smf")
                nc.vector.tensor_add(smf, qk_ps, biasF_t)
                mx = st_pool.tile([SPB * H, 1], f32)
                nc.vector.reduce_max(mx, smf, axis=AX.X)
                negmx = st_pool.tile([SPB * H, 1], f32)
                nc.vector.tensor_scalar_mul(negmx, mx, -1.0)
                ssum = st_pool.tile([SPB * H, 1], f32)
                nc.scalar.activation(
                    smf, smf, AF.Exp, bias=negmx, scale=1.0, accum_out=ssum
                )
                rinv = st_pool.tile([SPB * H, 1], f32)
                nc.vector.reciprocal(rinv, ssum)
                nc.vector.tensor_scalar_mul(smf, smf, rinv)
                # attn output: 8 diagonal-block DMAs (DMA has no partition
                # alignment restriction)
                for sl in range(SPB):
                    nc.sync.dma_start(
                        out=attn_d[b * SPB * H + sl * H : b * SPB * H + (sl + 1) * H, :],
                        in_=smf[sl * H : (sl + 1) * H, ts(sl, NBR)],
                    )
                # transpose prob chunks -> [(s',n), (s,h)]; gather diag cols
                a_t = a_pool.tile([128, RC, H], f32, tag="a")
                for rc in range(RC):
                    pT = tpps_pool.tile([128, SPB * H], f32, tag="tp")
                    nc.tensor.transpose(
                        pT, smf[:, ts(rc, 128)], ident_t[0 : SPB * H, 0 : SPB * H]
                    )
                    nc.vector.tensor_copy(a_t[0:64, rc, :], pT[0:64, ts(2 * rc, H)])
                    nc.vector.tensor_copy(
                        a_t[64:128, rc, :], pT[64:128, ts(2 * rc + 1, H)]
                    )

                # BD probs + AV matmuls accumulating into av_ps
                for h in range(H):
                    bd = bd_pool.tile([128, RC, NBR], bf16)
                    for c in range(RC):
                        nc.vector.tensor_scalar_mul(
                            bd[:, c, :], m0_t[:, b * RC + c, :], a_t[:, c, h : h + 1]
                        )
                    for c in range(RC):
                        nc.tensor.matmul(
                            av_ps[:, h, :],
                            v_t[:, c, ts(h, DK)],
                            bd[:, c, :],
                            start=(b == 0 and h == 0 and c == 0),
                            stop=(b == NBLK - 1 and c == RC - 1),
                        )

            # ---- tail: fc + residual + LN + merge MLP ----
            av_sb = tail.tile([128, H, S], f32r)
            nc.vector.tensor_copy(av_sb, av_ps)

            x0 = tail.tile([S, DM], f32)
            for half in range(2):
                fps = kps_pool.tile([128, 512], f32, tag="mm")
                for hd in range(8):
                    nc.tensor.matmul(
                        fps[0:S, :],
                        av_sb[:, hd, :],
                        fc_t[:, hd, ts(half, 512)],
                        start=(hd == 0),
                        stop=(hd == 7),
                    )
                nc.vector.tensor_add(
                    x0[:, ts(half, 512)], fps[0:S, :], qpfcb_t[:, ts(half, 512)]
                )

            # LayerNorm over the 1024 free dim
            x0_r = x0.rearrange("p (a b) -> p a b", b=512)
            stats = tail.tile([S, 2, nc.vector.BN_STATS_DIM], f32)
            for sub in range(2):
                nc.vector.bn_stats(stats[:, sub, :], x0_r[:, sub, :])
            mv = tail.tile([S, nc.vector.BN_AGGR_DIM], f32)
            nc.vector.bn_aggr(mv, stats)
            sd = tail.tile([S, 1], f32)
            nc.scalar.activation(sd, mv[:, 1:2], AF.Sqrt, bias=eps_t, scale=1.0)
            rstd = tail.tile([S, 1], f32)
            nc.vector.reciprocal(rstd, sd)
            xn = x0
            nc.vector.tensor_scalar(
                out=xn,
                in0=x0,
                scalar1=mv[:, 0:1],
                scalar2=rstd,
                op0=mybir.AluOpType.subtract,
                op1=mybir.AluOpType.mult,
            )

            # transpose xn into xcatT chunks 0..7 (src already in 8..9)
            for c in range(8):
                tp = tpps_pool.tile([128, S], f32, tag="tp")
                nc.tensor.transpose(tp, xn[:, ts(c, 128)], ident_t[0:S, 0:S])
                nc.vector.tensor_copy(xcatT_t[:, c, :], tp)

            # fc1 + relu
            h1ps = tpps_pool.tile([S, FEAT], f32, tag="tp")
            for c in range(10):
                nc.tensor.matmul(
                    h1ps,
                    xcatT_t[:, c, :],
                    fc1_t[:, c, :],
                    start=(c == 0),
                    stop=(c == 9),
                )
            h1 = tail.tile([S, FEAT], f32)
            nc.vector.tensor_add(h1, h1ps, fc1b_t)
            nc.scalar.activation(h1, h1, AF.Relu)

            # fc2
            h1T = tail.tile([128, 2, S], f32r)
            for c in range(2):
                tp = tpps_pool.tile([128, S], f32, tag="tp")
                nc.tensor.transpose(tp, h1[:, ts(c, 128)], ident_t[0:S, 0:S])
                nc.vector.tensor_copy(h1T[:, c, :], tp)
            zps = tpps_pool.tile([S, FEAT], f32, tag="tp")
            for c in range(2):
                nc.tensor.matmul(
                    zps,
                    h1T[:, c, :],
                    fc2_t[:, c, :],
                    start=(c == 0),
                    stop=(c == 1),
                )
            z_sb = tail.tile([S, FEAT], f32)
            nc.vector.tensor_add(z_sb, zps, fc2b_t)
            nc.sync.dma_start(out=z_d[:], in_=z_sb)

    _fix_multiwait(nc)
    return nc


def _host_prep(inp):
    """Build per-core input dicts from full inputs (host numpy only)."""
    f = np.float32
    src = np.asarray(inp["src"], f)
    src_t = np.asarray(inp["src_t"], f)
    src_p = np.asarray(inp["src_p"], f)
    seq = np.asarray(inp["seq"], f)
    seq_t = np.asarray(inp["seq_t"], f)
    seq_e = np.asarray(inp["seq_e"], f)
    seq_p = np.asarray(inp["seq_p"], f)
    mask = np.asarray(inp["mask"])
    fc_b = np.asarray(inp["fc_b"], f)

    k = np.concatenate([seq, seq_e, seq_t, seq_p], axis=2)          # [B,NGH,DM]
    q = np.concatenate([src, np.zeros_like(src), src_t, src_p], axis=2)

    wqT = np.ascontiguousarray(np.asarray(inp["w_qs"], f).T)
    wkT = np.ascontiguousarray(np.asarray(inp["w_ks"], f).T)
    wvT = np.ascontiguousarray(np.asarray(inp["w_vs"], f).T)
    fcT = np.ascontiguousarray(np.asarray(inp["fc_w"], f).T)
    ln_g = np.asarray(inp["ln_g"], f)
    ln_b = np.asarray(inp["ln_b"], f)
    fc1_w = np.asarray(inp["fc1_w"], f)
    # fold LayerNorm affine into fc1: h = fc1_w @ concat(xn*g+b, src) + b1
    fc1_w_mod = fc1_w.copy()
    fc1_w_mod[:, :DM] = fc1_w[:, :DM] * ln_g[None, :]
    fc1_b_mod = np.asarray(inp["fc1_b"], f) + fc1_w[:, :DM] @ ln_b
    fc1T = np.ascontiguousarray(fc1_w_mod.T)
    fc2T = np.ascontiguousarray(np.asarray(inp["fc2_w"], f).T)
    fc1b = np.ascontiguousarray(np.broadcast_to(fc1_b_mod, (S, FEAT)))
    fc2b = np.ascontiguousarray(np.broadcast_to(np.asarray(inp["fc2_b"], f), (S, FEAT)))

    m0 = np.zeros((128, 32, 64), ml_dtypes.bfloat16)
    for par in range(2):
        for c in range(32):
            m0[par * 64 : (par + 1) * 64, c, 2 * c + par] = 1.0

    scale = np.float32(DK ** -0.5)
    in_maps = []
    for core in range(NCORES):
        b0 = BPC * core
        k_c = k[b0 : b0 + BPC].reshape(ROWS, DM)
        q_c = q[b0 : b0 + BPC].reshape(S, DM)
        maskb = np.where(
            mask[b0 : b0 + BPC].reshape(S, NBR), f(-1e10), f(0.0)
        ).astype(f)
        bf = np.full((NBLK, SPB * H, BLK), f(-1e10), f)
        mb = maskb.reshape(NBLK, SPB, NBR)
        for sl in range(SPB):
            bf[:, sl * H : (sl + 1) * H, sl * NBR : (sl + 1) * NBR] = mb[
                :, sl, None, :
            ]
        in_maps.append(
            {
                "kt": np.ascontiguousarray(k_c.T),
                "qt": np.ascontiguousarray((q_c * scale).T),
                "qpfcb": np.ascontiguousarray(q_c + fc_b[None, :]),
                "srcT": np.ascontiguousarray(
                    src[b0 : b0 + BPC].reshape(S, FEAT).T
                ),
                "biasF": bf,
                "m0": m0,
                "wqT": wqT,
                "wkT": wkT,
                "wvT": wvT,
                "fcT": fcT,
                "fc1T": fc1T,
                "fc2T": fc2T,
                "fc1b": fc1b,
                "fc2b": fc2b,
            }
        )
    return in_maps


def _install_trace_hook():
    """Register the NTFF profile hook (missing antenv.axon_hooks in image)."""
    import sys
    import types

    if "antenv.axon_hooks" in sys.modules:
        return
    import antenv

    mod = types.ModuleType("antenv.axon_hooks")
    _hook = [None]
    mod.set_axon_ntff_profile_hook = lambda h: _hook.__setitem__(0, h)
    mod.get_axon_ntff_profile_hook = lambda: _hook[0]
    sys.modules["antenv.axon_hooks"] = mod
    antenv.axon_hooks = mod
    try:
        from trn_agent_boot.trn_boot import _ntff_profile_via_ctypes

        h = _ntff_profile_via_ctypes("/opt/axon/libaxon_pjrt.so")
        if h is not None:
            mod.set_axon_ntff_profile_hook(h)
    except Exception:
        pass


def kernel(**inputs):
    global LAST_EXEC_NS
    from concourse.bass_utils import run_bass_kernel_spmd

    trace = bool(os.environ.get("BASS_KERNEL_TRACE"))
    if trace:
        _install_trace_hook()

    if "nc" not in _CACHE:
        _CACHE["nc"] = _build_nc()
    nc = _CACHE["nc"]

    in_maps = _host_prep(inputs)
    kwargs = {}
    if trace:
        kwargs["trace"] = True
        td = os.environ.get("BASS_KERNEL_TRACE_DIR")
        if td:
            os.makedirs(td, exist_ok=True)
            kwargs["tmpdir"] = td
    res = run_bass_kernel_spmd(nc, in_maps, list(range(NCORES)), **kwargs)
    LAST_EXEC_NS = res.exec_time_ns

    z = np.stack([res.results[i]["z"] for i in range(NCORES)]).reshape(B, NS, FEAT)
    attn = np.stack([res.results[i]["attn"] for i in range(NCORES)]).reshape(
        B, NS, H, NBR
    )
    return z, attn


# revision 23
# speedup vs baseline: 1.1466x; 1.1466x over previous
"""Trainium2 Bass kernel for the TGAT-style AttnModel (gnn_message_passing).

Contract: kernel(**inputs) takes FULL unsharded numpy inputs (as produced by
setup_inputs()) and returns the FULL output tuple (z, attn).

Strategy: pure data parallel over batch B=16 -> 2 batches per NeuronCore
(8 cores). Per core: 64 sources, 4096 neighbor rows. All projections run as
fp32r matmuls on the PE; attention uses a block-diagonal-Q trick for QK and a
block-diagonal-probs (BD) trick for attn@V; softmax in fp32 on DVE/ACT.
Host-side prep does the concats/transposes (pure data movement).
"""

import os
import numpy as np

import ml_dtypes

# ---- model constants (hardcoded; kernel.py must be self-contained) ----
B, NS, NGH, NBR = 16, 32, 2048, 64
FEAT = 256
DM, H, DK = 1024, 8, 128
LN_EPS = 1e-5

NCORES = 8
BPC = B // NCORES          # 2 batches per core
S = BPC * NS               # 64 sources per core
ROWS = BPC * NGH           # 4096 k-rows per core
NBLK = 8
BLK = ROWS // NBLK         # 512 rows per block
SPB = S // NBLK            # 8 sources per block
RC = BLK // 128            # 4 row-chunks of 128 per block

_CACHE = {}
LAST_EXEC_NS = None


def _patch_drain():
    """walrus in this container rejects instructions with >1 sync wait on the
    Drain ctrl struct; split the TileContext tail-drain waits into one drain
    per semaphore."""
    import concourse.tile as tile
    from concourse import mybir
    from concourse.vector_clock import ScopedClock

    if getattr(tile.TileContext, "_drain_patched", False):
        return

    def _drain_and_barrier(self, tick_clock, wait_clock):
        nc = self.nc
        drain_inst = nc.sync.drain()
        wait_clock.add_sem_waits(
            drain_inst.ins, ScopedClock({None: tick_clock.global_clock})
        )
        ri = drain_inst.ins
        waits = list(ri.sync_info.on_wait)
        ri.sync_info = mybir.SyncInfo(on_wait=waits[:1], on_update=[])
        for w in waits[1:]:
            d2 = nc.sync.drain()
            d2.ins.sync_info = mybir.SyncInfo(on_wait=[w], on_update=[])
        nc.all_engine_barrier()
        popped = nc._tile_sem_poison_stack.pop()
        assert popped is self._sem_poison
        nc.clear_and_free_semaphores(list(self.sems.allocated().values()))
        nc.all_engine_barrier()

    tile.TileContext._drain_and_barrier = _drain_and_barrier
    tile.TileContext._drain_patched = True


def _fix_multiwait(nc):
    """walrus in this container accepts very few sync commands per
    instruction (1 wait on NOP/Drain ctrl; a wait+update on engine ctrl).
    Conservatively rewrite every block so each instruction carries at most
    ONE wait: excess waits move to same-engine NoOp carriers inserted
    immediately before the instruction — identical semantics (same engine,
    same program position), so no scheduling or deadlock risk."""
    from concourse import mybir

    ctr = [0]
    for f in nc.m.functions:
        for bb in f.blocks:
            out = []
            changed = False
            for inst in bb.instructions:
                si = inst.sync_info
                if si is not None and len(si.on_wait) > 1:
                    waits = list(si.on_wait)
                    for w in waits[:-1]:
                        ctr[0] += 1
                        nop = mybir.InstNoOp(
                            name=f"I-wfix-{ctr[0]}", ins=[], outs=[]
                        )
                        nop.engine = inst.engine
                        nop.sync_info = mybir.SyncInfo(
                            on_wait=[w], on_update=[]
                        )
                        out.append(nop)
                    inst.sync_info = mybir.SyncInfo(
                        on_wait=[waits[-1]], on_update=list(si.on_update)
                    )
                    changed = True
                out.append(inst)
            if changed:
                bb.instructions = out


def _build_nc():
    import concourse.bass as bass
    import concourse.tile as tile
    import concourse.tile_utils as tile_utils
    from concourse import mybir
    from concourse.bass import ts
    from concourse.masks import make_identity

    _patch_drain()
    # stale 192KB cap; cayman has 208KB usable per partition
    tile_utils.max_sbuf_usage = 207 * 1024

    f32 = mybir.dt.float32
    f32r = mybir.dt.float32r
    bf16 = mybir.dt.bfloat16
    AX = mybir.AxisListType
    AF = mybir.ActivationFunctionType

    nc = bass.Bass(target_bir_lowering=False)

    # ---- DRAM parameters (per-core views, host-prepared) ----
    kt_d = nc.declare_dram_parameter("kt", [DM, ROWS], bf16, isOutput=False)
    qt_d = nc.declare_dram_parameter("qt", [DM, S], f32r, isOutput=False)
    qpfcb_d = nc.declare_dram_parameter("qpfcb", [S, DM], f32, isOutput=False)
    srcT_d = nc.declare_dram_parameter("srcT", [FEAT, S], f32r, isOutput=False)
    biasF_d = nc.declare_dram_parameter("biasF", [NBLK, SPB * H, BLK], f32, isOutput=False)
    m0_d = nc.declare_dram_parameter("m0", [128, 32, 64], bf16, isOutput=False)
    wqT_d = nc.declare_dram_parameter("wqT", [DM, DM], f32r, isOutput=False)
    wkT_d = nc.declare_dram_parameter("wkT", [DM, DM], bf16, isOutput=False)
    wvT_d = nc.declare_dram_parameter("wvT", [DM, DM], bf16, isOutput=False)
    fcT_d = nc.declare_dram_parameter("fcT", [DM, DM], f32r, isOutput=False)
    fc1T_d = nc.declare_dram_parameter("fc1T", [DM + FEAT, FEAT], f32r, isOutput=False)
    fc2T_d = nc.declare_dram_parameter("fc2T", [FEAT, FEAT], f32r, isOutput=False)
    fc1b_d = nc.declare_dram_parameter("fc1b", [S, FEAT], f32, isOutput=False)
    fc2b_d = nc.declare_dram_parameter("fc2b", [S, FEAT], f32, isOutput=False)
    z_d = nc.declare_dram_parameter("z", [S, FEAT], f32, isOutput=True)
    attn_d = nc.declare_dram_parameter("attn", [S * H, NBR], f32, isOutput=True)

    with tile.TileContext(nc) as tc:
        with (
            tc.tile_pool(name="const", bufs=1) as const,
            tc.tile_pool(name="kt", bufs=2) as ktp_pool,
            tc.tile_pool(name="kproj", bufs=2) as kproj_pool,
            tc.tile_pool(name="v", bufs=2) as v_pool,
            tc.tile_pool(name="bdq", bufs=2) as bdq_pool,
            tc.tile_pool(name="sm", bufs=2) as sm_pool,
            tc.tile_pool(name="bf", bufs=2) as bf_pool,
            tc.tile_pool(name="st", bufs=8) as st_pool,
            tc.tile_pool(name="a", bufs=2) as a_pool,
            tc.tile_pool(name="bd", bufs=2) as bd_pool,
            tc.tile_pool(name="tail", bufs=1) as tail,
            tc.tile_pool(name="kps", bufs=2, space="PSUM") as kps_pool,
            tc.tile_pool(name="vps", bufs=2, space="PSUM") as vps_pool,
            tc.tile_pool(name="qkps", bufs=1, space="PSUM") as qkps_pool,
            tc.tile_pool(name="avps", bufs=1, space="PSUM") as avps_pool,
            tc.tile_pool(name="tpps", bufs=2, space="PSUM") as tpps_pool,
        ):
            # ---- constants / weights ----
            wk_t = const.tile([128, 8, DM], bf16)
            wv_t = const.tile([128, 8, DM], bf16)
            fc_t = const.tile([128, 8, DM], f32r)  # holds wqT first, fcT later
            fc1_t = const.tile([128, 10, FEAT], f32r)
            fc2_t = const.tile([128, 2, FEAT], f32r)
            m0_t = const.tile([128, 32, 64], bf16)
            qpfcb_t = const.tile([S, DM], f32)
            fc1b_t = const.tile([S, FEAT], f32)
            fc2b_t = const.tile([S, FEAT], f32)
            xcatT_t = const.tile([128, 10, S], f32r)
            qproj_t = const.tile([128, 8, S], f32)
            ident_t = const.tile([128, 128], f32)
            eps_t = const.tile([S, 1], f32)
            zero64_t = const.tile([128, SPB * H], f32)

            # startup-ordered loads: wk -> kt(b0)+biasF(b0) -> qt+wqT -> wv -> m0
            for i in range(8):
                nc.sync.dma_start(out=wk_t[:, i, :], in_=wkT_d[ts(i, 128), :])
            kt0_t = ktp_pool.tile([128, 8, BLK], bf16, tag="kt_t")
            for i in range(8):
                nc.sync.dma_start(out=kt0_t[:, i, :], in_=kt_d[ts(i, 128), ts(0, BLK)])
            biasF0_t = bf_pool.tile([SPB * H, BLK], f32, tag="biasF_t")
            nc.sync.dma_start(out=biasF0_t, in_=biasF_d[0])
            qt_t = const.tile([128, 8, S], f32r)
            for i in range(8):
                nc.sync.dma_start(out=qt_t[:, i, :], in_=qt_d[ts(i, 128), :])
                nc.sync.dma_start(out=fc_t[:, i, :], in_=wqT_d[ts(i, 128), :])
            for i in range(8):
                nc.sync.dma_start(out=wv_t[:, i, :], in_=wvT_d[ts(i, 128), :])
            nc.sync.dma_start(out=m0_t[:], in_=m0_d[:])
            make_identity(nc, ident_t)
            nc.vector.memset(eps_t, LN_EPS)
            nc.vector.memset(zero64_t, 0.0)

            # ---- K-projection of block 0 (PE starts as soon as wk+kt0 land)
            ktproj0_t = kproj_pool.tile([128, 8, BLK], f32r, tag="ktproj_t")
            for o in range(8):
                ps = kps_pool.tile([128, 512], f32, tag="mm")
                for i in range(8):
                    nc.tensor.matmul(
                        ps,
                        wk_t[:, i, ts(o, 128)],
                        kt0_t[:, i, :],
                        start=(i == 0),
                        stop=(i == 7),
                    )
                nc.vector.tensor_copy(ktproj0_t[:, o, :], ps)

            # ---- Q projection: qproj[(h,d) chunk h][d, s] ----
            # fc_t currently holds wqT; it is re-loaded with fcT after the loop
            for o in range(8):
                qps = kps_pool.tile([128, 512], f32, tag="mm")
                for i in range(8):
                    nc.tensor.matmul(
                        qps[:, 0:S],
                        fc_t[:, i, ts(o, 128)],
                        qt_t[:, i, :],
                        start=(i == 0),
                        stop=(i == 7),
                    )
                nc.vector.tensor_copy(qproj_t[:, o, :], qps[:, 0:S])

            # ---- persistent AV accumulator: [d, h, s] (one PSUM bank) ----
            av_ps = avps_pool.tile([128, H, S], f32)

            for b in range(NBLK):
                if b == 0:
                    kt_t = kt0_t
                    biasF_t = biasF0_t
                    ktproj_t = ktproj0_t
                else:
                    # load k^T block
                    kt_t = ktp_pool.tile([128, 8, BLK], bf16, tag="kt_t")
                    for i in range(8):
                        nc.sync.dma_start(
                            out=kt_t[:, i, :], in_=kt_d[ts(i, 128), ts(b, BLK)]
                        )
                    biasF_t = bf_pool.tile([SPB * H, BLK], f32, tag="biasF_t")
                    nc.sync.dma_start(out=biasF_t, in_=biasF_d[b])

                    # K-projection (transposed out): ktp[(h,d) chunk][d, rows]
                    ktproj_t = kproj_pool.tile([128, 8, BLK], f32r, tag="ktproj_t")
                    for o in range(8):
                        ps = kps_pool.tile([128, 512], f32, tag="mm")
                        for i in range(8):
                            nc.tensor.matmul(
                                ps,
                                wk_t[:, i, ts(o, 128)],
                                kt_t[:, i, :],
                                start=(i == 0),
                                stop=(i == 7),
                            )
                        nc.vector.tensor_copy(ktproj_t[:, o, :], ps)

                # V-projection (normal out, bf16): v[rowchunk][row, (h,d)]
                v_t = v_pool.tile([128, RC, DM], bf16)
                for rc in range(RC):
                    for half in range(2):
                        ps = vps_pool.tile([128, 512], f32, tag="vmm")
                        for i in range(8):
                            nc.tensor.matmul(
                                ps,
                                kt_t[:, i, ts(rc, 128)],
                                wv_t[:, i, ts(half, 512)],
                                start=(i == 0),
                                stop=(i == 7),
                            )
                        nc.vector.tensor_copy(v_t[:, rc, ts(half, 512)], ps)

                # QK via block-diagonal Q: out[(s,h), rows]
                qk_ps = qkps_pool.tile([SPB * H, BLK], f32)
                for h in range(H):
                    bdq = bdq_pool.tile([128, SPB * H], f32r)
                    bdq_r = bdq.rearrange("p (s h) -> p s h", h=H)
                    nc.vector.tensor_copy(bdq, zero64_t)
                    nc.vector.tensor_copy(bdq_r[:, :, h], qproj_t[:, h, ts(b, SPB)])
                    nc.tensor.matmul(
                        qk_ps,
                        bdq,
                        ktproj_t[:, h, :],
                        start=(h == 0),
                        stop=(h == 7),
                    )

                # mask/bias over the FULL row (off-diagonal cols get -1e10,
                # so softmax over 512 cols == per-source softmax over 64)
                smf = sm_pool.tile([SPB * H, BLK], f32, tag="smf")
                nc.vector.tensor_add(smf, qk_ps, biasF_t)
                mx = st_pool.tile([SPB * H, 1], f32)
                nc.vector.reduce_max(mx, smf, axis=AX.X)
                negmx = st_pool.tile([SPB * H, 1], f32)
                nc.vector.tensor_scalar_mul(negmx, mx, -1.0)
                ssum = st_pool.tile([SPB * H, 1], f32)
                nc.scalar.activation(
                    smf, smf, AF.Exp, bias=negmx, scale=1.0, accum_out=ssum
                )
                rinv = st_pool.tile([SPB * H, 1], f32)
                nc.vector.reciprocal(rinv, ssum)
                nc.vector.tensor_scalar_mul(smf, smf, rinv)
                # attn output: 8 diagonal-block DMAs (DMA has no partition
                # alignment restriction)
                for sl in range(SPB):
                    nc.sync.dma_start(
                        out=attn_d[b * SPB * H + sl * H : b * SPB * H + (sl + 1) * H, :],
                        in_=smf[sl * H : (sl + 1) * H, ts(sl, NBR)],
                    )
                # transpose prob chunks -> [(s',n), (s,h)]; gather diag cols
                a_t = a_pool.tile([128, RC, H], f32, tag="a")
                for rc in range(RC):
                    pT = tpps_pool.tile([128, SPB * H], f32, tag="tp")
                    nc.tensor.transpose(
                        pT, smf[:, ts(rc, 128)], ident_t[0 : SPB * H, 0 : SPB * H]
                    )
                    nc.vector.tensor_copy(a_t[0:64, rc, :], pT[0:64, ts(2 * rc, H)])
                    nc.vector.tensor_copy(
                        a_t[64:128, rc, :], pT[64:128, ts(2 * rc + 1, H)]
                    )

                # BD probs + AV matmuls accumulating into av_ps
                for h in range(H):
                    bd = bd_pool.tile([128, RC, NBR], bf16)
                    for c in range(RC):
                        nc.vector.tensor_scalar_mul(
                            bd[:, c, :], m0_t[:, b * RC + c, :], a_t[:, c, h : h + 1]
                        )
                    for c in range(RC):
                        nc.tensor.matmul(
                            av_ps[:, h, :],
                            v_t[:, c, ts(h, DK)],
                            bd[:, c, :],
                            start=(b == 0 and h == 0 and c == 0),
                            stop=(b == NBLK - 1 and c == RC - 1),
                        )

            # ---- tail consts (emitted late so startup DMA stays lean) ----
            # overwrite fc_t (held wqT) with fcT; WAR handled by Tile
            for i in range(8):
                nc.sync.dma_start(out=fc_t[:, i, :], in_=fcT_d[ts(i, 128), :])
            for i in range(10):
                nc.sync.dma_start(out=fc1_t[:, i, :], in_=fc1T_d[ts(i, 128), :])
            for i in range(2):
                nc.sync.dma_start(out=fc2_t[:, i, :], in_=fc2T_d[ts(i, 128), :])
                nc.sync.dma_start(out=xcatT_t[:, 8 + i, :], in_=srcT_d[ts(i, 128), :])
            nc.sync.dma_start(out=qpfcb_t[:], in_=qpfcb_d[:])
            nc.sync.dma_start(out=fc1b_t[:], in_=fc1b_d[:])
            nc.sync.dma_start(out=fc2b_t[:], in_=fc2b_d[:])

            # ---- tail: fc + residual + LN + merge MLP ----
            av_sb = tail.tile([128, H, S], f32r)
            nc.vector.tensor_copy(av_sb, av_ps)

            x0 = tail.tile([S, DM], f32)
            for half in range(2):
                fps = kps_pool.tile([128, 512], f32, tag="mm")
                for hd in range(8):
                    nc.tensor.matmul(
                        fps[0:S, :],
                        av_sb[:, hd, :],
                        fc_t[:, hd, ts(half, 512)],
                        start=(hd == 0),
                        stop=(hd == 7),
                    )
                nc.vector.tensor_add(
                    x0[:, ts(half, 512)], fps[0:S, :], qpfcb_t[:, ts(half, 512)]
                )

            # LayerNorm over the 1024 free dim
            x0_r = x0.rearrange("p (a b) -> p a b", b=512)
            stats = tail.tile([S, 2, nc.vector.BN_STATS_DIM], f32)
            for sub in range(2):
                nc.vector.bn_stats(stats[:, sub, :], x0_r[:, sub, :])
            mv = tail.tile([S, nc.vector.BN_AGGR_DIM], f32)
            nc.vector.bn_aggr(mv, stats)
            sd = tail.tile([S, 1], f32)
            nc.scalar.activation(sd, mv[:, 1:2], AF.Sqrt, bias=eps_t, scale=1.0)
            rstd = tail.tile([S, 1], f32)
            nc.vector.reciprocal(rstd, sd)
            xn = x0
            nc.vector.tensor_scalar(
                out=xn,
                in0=x0,
                scalar1=mv[:, 0:1],
                scalar2=rstd,
                op0=mybir.AluOpType.subtract,
                op1=mybir.AluOpType.mult,
            )

            # transpose xn into xcatT chunks 0..7 (src already in 8..9)
            for c in range(8):
                tp = tpps_pool.tile([128, S], f32, tag="tp")
                nc.tensor.transpose(tp, xn[:, ts(c, 128)], ident_t[0:S, 0:S])
                nc.vector.tensor_copy(xcatT_t[:, c, :], tp)

            # fc1 + relu
            h1ps = tpps_pool.tile([S, FEAT], f32, tag="tp")
            for c in range(10):
                nc.tensor.matmul(
                    h1ps,
                    xcatT_t[:, c, :],
                    fc1_t[:, c, :],
                    start=(c == 0),
                    stop=(c == 9),
                )
            h1 = tail.tile([S, FEAT], f32)
            nc.vector.tensor_add(h1, h1ps, fc1b_t)
            nc.scalar.activation(h1, h1, AF.Relu)

            # fc2
            h1T = tail.tile([128, 2, S], f32r)
            for c in range(2):
                tp = tpps_pool.tile([128, S], f32, tag="tp")
                nc.tensor.transpose(tp, h1[:, ts(c, 128)], ident_t[0:S, 0:S])
                nc.vector.tensor_copy(h1T[:, c, :], tp)
            zps = tpps_pool.tile([S, FEAT], f32, tag="tp")
            for c in range(2):
                nc.tensor.matmul(
                    zps,
                    h1T[:, c, :],
                    fc2_t[:, c, :],
                    start=(c == 0),
                    stop=(c == 1),
                )
            z_sb = tail.tile([S, FEAT], f32)
            nc.vector.tensor_add(z_sb, zps, fc2b_t)
            nc.sync.dma_start(out=z_d[:], in_=z_sb)

    _fix_multiwait(nc)
    return nc


def _host_prep(inp):
    """Build per-core input dicts from full inputs (host numpy only)."""
    f = np.float32
    src = np.asarray(inp["src"], f)
    src_t = np.asarray(inp["src_t"], f)
    src_p = np.asarray(inp["src_p"], f)
    seq = np.asarray(inp["seq"], f)
    seq_t = np.asarray(inp["seq_t"], f)
    seq_e = np.asarray(inp["seq_e"], f)
    seq_p = np.asarray(inp["seq_p"], f)
    mask = np.asarray(inp["mask"])
    fc_b = np.asarray(inp["fc_b"], f)

    k = np.concatenate([seq, seq_e, seq_t, seq_p], axis=2)          # [B,NGH,DM]
    q = np.concatenate([src, np.zeros_like(src), src_t, src_p], axis=2)

    wqT = np.ascontiguousarray(np.asarray(inp["w_qs"], f).T)
    wkT = np.ascontiguousarray(np.asarray(inp["w_ks"], f).T.astype(ml_dtypes.bfloat16))
    wvT = np.ascontiguousarray(np.asarray(inp["w_vs"], f).T.astype(ml_dtypes.bfloat16))
    fcT = np.ascontiguousarray(np.asarray(inp["fc_w"], f).T)
    ln_g = np.asarray(inp["ln_g"], f)
    ln_b = np.asarray(inp["ln_b"], f)
    fc1_w = np.asarray(inp["fc1_w"], f)
    # fold LayerNorm affine into fc1: h = fc1_w @ concat(xn*g+b, src) + b1
    fc1_w_mod = fc1_w.copy()
    fc1_w_mod[:, :DM] = fc1_w[:, :DM] * ln_g[None, :]
    fc1_b_mod = np.asarray(inp["fc1_b"], f) + fc1_w[:, :DM] @ ln_b
    fc1T = np.ascontiguousarray(fc1_w_mod.T)
    fc2T = np.ascontiguousarray(np.asarray(inp["fc2_w"], f).T)
    fc1b = np.ascontiguousarray(np.broadcast_to(fc1_b_mod, (S, FEAT)))
    fc2b = np.ascontiguousarray(np.broadcast_to(np.asarray(inp["fc2_b"], f), (S, FEAT)))

    m0 = np.zeros((128, 32, 64), ml_dtypes.bfloat16)
    for par in range(2):
        for c in range(32):
            m0[par * 64 : (par + 1) * 64, c, 2 * c + par] = 1.0

    scale = np.float32(DK ** -0.5)
    in_maps = []
    for core in range(NCORES):
        b0 = BPC * core
        k_c = k[b0 : b0 + BPC].reshape(ROWS, DM)
        q_c = q[b0 : b0 + BPC].reshape(S, DM)
        maskb = np.where(
            mask[b0 : b0 + BPC].reshape(S, NBR), f(-1e10), f(0.0)
        ).astype(f)
        bf = np.full((NBLK, SPB * H, BLK), f(-1e10), f)
        mb = maskb.reshape(NBLK, SPB, NBR)
        for sl in range(SPB):
            bf[:, sl * H : (sl + 1) * H, sl * NBR : (sl + 1) * NBR] = mb[
                :, sl, None, :
            ]
        in_maps.append(
            {
                "kt": np.ascontiguousarray(k_c.T.astype(ml_dtypes.bfloat16)),
                "qt": np.ascontiguousarray((q_c * scale).T),
                "qpfcb": np.ascontiguousarray(q_c + fc_b[None, :]),
                "srcT": np.ascontiguousarray(
                    src[b0 : b0 + BPC].reshape(S, FEAT).T
                ),
                "biasF": bf,
                "m0": m0,
                "wqT": wqT,
                "wkT": wkT,
                "wvT": wvT,
                "fcT": fcT,
                "fc1T": fc1T,
                "fc2T": fc2T,
                "fc1b": fc1b,
                "fc2b": fc2b,
            }
        )
    return in_maps


def _install_trace_hook():
    """Register the NTFF profile hook (missing antenv.axon_hooks in image)."""
    import sys
    import types

    if "antenv.axon_hooks" in sys.modules:
        return
    import antenv

    mod = types.ModuleType("antenv.axon_hooks")
    _hook = [None]
    mod.set_axon_ntff_profile_hook = lambda h: _hook.__setitem__(0, h)
    mod.get_axon_ntff_profile_hook = lambda: _hook[0]
    sys.modules["antenv.axon_hooks"] = mod
    antenv.axon_hooks = mod
    try:
        from trn_agent_boot.trn_boot import _ntff_profile_via_ctypes

        h = _ntff_profile_via_ctypes("/opt/axon/libaxon_pjrt.so")
        if h is not None:
            mod.set_axon_ntff_profile_hook(h)
    except Exception:
        pass


def kernel(**inputs):
    global LAST_EXEC_NS
    from concourse.bass_utils import run_bass_kernel_spmd

    trace = bool(os.environ.get("BASS_KERNEL_TRACE"))
    if trace:
        _install_trace_hook()

    if "nc" not in _CACHE:
        _CACHE["nc"] = _build_nc()
    nc = _CACHE["nc"]

    in_maps = _host_prep(inputs)
    kwargs = {}
    if trace:
        kwargs["trace"] = True
        td = os.environ.get("BASS_KERNEL_TRACE_DIR")
        if td:
            os.makedirs(td, exist_ok=True)
            kwargs["tmpdir"] = td
    res = run_bass_kernel_spmd(nc, in_maps, list(range(NCORES)), **kwargs)
    LAST_EXEC_NS = res.exec_time_ns

    z = np.stack([res.results[i]["z"] for i in range(NCORES)]).reshape(B, NS, FEAT)
    attn = np.stack([res.results[i]["attn"] for i in range(NCORES)]).reshape(
        B, NS, H, NBR
    )
    return z, attn


# revision 27
# speedup vs baseline: 1.1830x; 1.0318x over previous
"""Trainium2 Bass kernel for the TGAT-style AttnModel (gnn_message_passing).

Contract: kernel(**inputs) takes FULL unsharded numpy inputs (as produced by
setup_inputs()) and returns the FULL output tuple (z, attn).

Strategy: pure data parallel over batch B=16 -> 2 batches per NeuronCore
(8 cores). Per core: 64 sources, 4096 neighbor rows. All projections run as
fp32r matmuls on the PE; attention uses a block-diagonal-Q trick for QK and a
block-diagonal-probs (BD) trick for attn@V; softmax in fp32 on DVE/ACT.
Host-side prep does the concats/transposes (pure data movement).
"""

import os
import numpy as np

import ml_dtypes

# ---- model constants (hardcoded; kernel.py must be self-contained) ----
B, NS, NGH, NBR = 16, 32, 2048, 64
FEAT = 256
DM, H, DK = 1024, 8, 128
LN_EPS = 1e-5

NCORES = 8
BPC = B // NCORES          # 2 batches per core
S = BPC * NS               # 64 sources per core
ROWS = BPC * NGH           # 4096 k-rows per core
NBLK = 8
BLK = ROWS // NBLK         # 512 rows per block
SPB = S // NBLK            # 8 sources per block
RC = BLK // 128            # 4 row-chunks of 128 per block

_CACHE = {}
LAST_EXEC_NS = None


def _patch_drain():
    """walrus in this container rejects instructions with >1 sync wait on the
    Drain ctrl struct; split the TileContext tail-drain waits into one drain
    per semaphore."""
    import concourse.tile as tile
    from concourse import mybir
    from concourse.vector_clock import ScopedClock

    if getattr(tile.TileContext, "_drain_patched", False):
        return

    def _drain_and_barrier(self, tick_clock, wait_clock):
        nc = self.nc
        drain_inst = nc.sync.drain()
        wait_clock.add_sem_waits(
            drain_inst.ins, ScopedClock({None: tick_clock.global_clock})
        )
        ri = drain_inst.ins
        waits = list(ri.sync_info.on_wait)
        ri.sync_info = mybir.SyncInfo(on_wait=waits[:1], on_update=[])
        for w in waits[1:]:
            d2 = nc.sync.drain()
            d2.ins.sync_info = mybir.SyncInfo(on_wait=[w], on_update=[])
        nc.all_engine_barrier()
        popped = nc._tile_sem_poison_stack.pop()
        assert popped is self._sem_poison
        nc.clear_and_free_semaphores(list(self.sems.allocated().values()))
        nc.all_engine_barrier()

    tile.TileContext._drain_and_barrier = _drain_and_barrier
    tile.TileContext._drain_patched = True


def _fix_multiwait(nc):
    """walrus in this container accepts very few sync commands per
    instruction (1 wait on NOP/Drain ctrl; a wait+update on engine ctrl).
    Conservatively rewrite every block so each instruction carries at most
    ONE wait: excess waits move to same-engine NoOp carriers inserted
    immediately before the instruction — identical semantics (same engine,
    same program position), so no scheduling or deadlock risk."""
    from concourse import mybir

    ctr = [0]
    for f in nc.m.functions:
        for bb in f.blocks:
            out = []
            changed = False
            for inst in bb.instructions:
                si = inst.sync_info
                if si is not None and len(si.on_wait) > 1:
                    waits = list(si.on_wait)
                    for w in waits[:-1]:
                        ctr[0] += 1
                        nop = mybir.InstNoOp(
                            name=f"I-wfix-{ctr[0]}", ins=[], outs=[]
                        )
                        nop.engine = inst.engine
                        nop.sync_info = mybir.SyncInfo(
                            on_wait=[w], on_update=[]
                        )
                        out.append(nop)
                    inst.sync_info = mybir.SyncInfo(
                        on_wait=[waits[-1]], on_update=list(si.on_update)
                    )
                    changed = True
                out.append(inst)
            if changed:
                bb.instructions = out


def _build_nc():
    import concourse.bass as bass
    import concourse.tile as tile
    import concourse.tile_utils as tile_utils
    from concourse import mybir
    from concourse.bass import ts
    from concourse.masks import make_identity

    _patch_drain()
    # stale 192KB cap; cayman has 208KB usable per partition
    tile_utils.max_sbuf_usage = 207 * 1024

    f32 = mybir.dt.float32
    f32r = mybir.dt.float32r
    bf16 = mybir.dt.bfloat16
    AX = mybir.AxisListType
    AF = mybir.ActivationFunctionType

    nc = bass.Bass(target_bir_lowering=False)

    # ---- DRAM parameters (per-core views, host-prepared) ----
    kt_d = nc.declare_dram_parameter("kt", [DM, ROWS], bf16, isOutput=False)
    qt_d = nc.declare_dram_parameter("qt", [DM, S], bf16, isOutput=False)
    qpfcb_d = nc.declare_dram_parameter("qpfcb", [S, DM], f32, isOutput=False)
    srcT_d = nc.declare_dram_parameter("srcT", [FEAT, S], f32r, isOutput=False)
    biasF_d = nc.declare_dram_parameter("biasF", [NBLK, SPB * H, BLK], f32, isOutput=False)
    m0_d = nc.declare_dram_parameter("m0", [128, 32, 64], bf16, isOutput=False)
    wqT_d = nc.declare_dram_parameter("wqT", [DM, DM], bf16, isOutput=False)
    wkT_d = nc.declare_dram_parameter("wkT", [DM, DM], bf16, isOutput=False)
    wvT_d = nc.declare_dram_parameter("wvT", [DM, DM], bf16, isOutput=False)
    fcT_d = nc.declare_dram_parameter("fcT", [DM, DM], f32r, isOutput=False)
    fc1T_d = nc.declare_dram_parameter("fc1T", [DM + FEAT, FEAT], f32r, isOutput=False)
    fc2T_d = nc.declare_dram_parameter("fc2T", [FEAT, FEAT], f32r, isOutput=False)
    fc1b_d = nc.declare_dram_parameter("fc1b", [S, FEAT], f32, isOutput=False)
    fc2b_d = nc.declare_dram_parameter("fc2b", [S, FEAT], f32, isOutput=False)
    z_d = nc.declare_dram_parameter("z", [S, FEAT], f32, isOutput=True)
    attn_d = nc.declare_dram_parameter("attn", [S * H, NBR], f32, isOutput=True)

    with tile.TileContext(nc) as tc:
        with (
            tc.tile_pool(name="const", bufs=1) as const,
            tc.tile_pool(name="kt", bufs=2) as ktp_pool,
            tc.tile_pool(name="kproj", bufs=2) as kproj_pool,
            tc.tile_pool(name="v", bufs=2) as v_pool,
            tc.tile_pool(name="bdq", bufs=2) as bdq_pool,
            tc.tile_pool(name="sm", bufs=2) as sm_pool,
            tc.tile_pool(name="bf", bufs=2) as bf_pool,
            tc.tile_pool(name="st", bufs=8) as st_pool,
            tc.tile_pool(name="a", bufs=2) as a_pool,
            tc.tile_pool(name="bd", bufs=2) as bd_pool,
            tc.tile_pool(name="tail", bufs=1) as tail,
            tc.tile_pool(name="kps", bufs=2, space="PSUM") as kps_pool,
            tc.tile_pool(name="vps", bufs=2, space="PSUM") as vps_pool,
            tc.tile_pool(name="qkps", bufs=1, space="PSUM") as qkps_pool,
            tc.tile_pool(name="avps", bufs=1, space="PSUM") as avps_pool,
            tc.tile_pool(name="tpps", bufs=2, space="PSUM") as tpps_pool,
        ):
            # ---- constants / weights ----
            wk_t = const.tile([128, 8, DM], bf16)
            wv_t = const.tile([128, 8, DM], bf16)
            fc_t = const.tile([128, 8, DM], f32r)  # holds wqT first, fcT later
            fc1_t = const.tile([128, 10, FEAT], f32r)
            fc2_t = const.tile([128, 2, FEAT], f32r)
            m0_t = const.tile([128, 32, 64], bf16)
            qpfcb_t = const.tile([S, DM], f32)
            fc1b_t = const.tile([S, FEAT], f32)
            fc2b_t = const.tile([S, FEAT], f32)
            xcatT_t = const.tile([128, 10, S], f32r)
            qproj_t = const.tile([128, 8, S], f32)
            ident_t = const.tile([128, 128], f32)
            eps_t = const.tile([S, 1], f32)
            zero64_t = const.tile([128, SPB * H], f32)

            # startup-ordered loads: wk -> kt(b0)+biasF(b0) -> qt+wqT -> wv -> m0
            for i in range(8):
                nc.sync.dma_start(out=wk_t[:, i, :], in_=wkT_d[ts(i, 128), :])
            kt0_t = ktp_pool.tile([128, 8, BLK], bf16, tag="kt_t")
            for i in range(8):
                nc.sync.dma_start(out=kt0_t[:, i, :], in_=kt_d[ts(i, 128), ts(0, BLK)])
            biasF0_t = bf_pool.tile([SPB * H, BLK], f32, tag="biasF_t")
            nc.sync.dma_start(out=biasF0_t, in_=biasF_d[0])
            qt_t = const.tile([128, 8, S], bf16)
            wq_t = const.tile([128, 8, DM], bf16)
            for i in range(8):
                nc.sync.dma_start(out=qt_t[:, i, :], in_=qt_d[ts(i, 128), :])
                nc.sync.dma_start(out=wq_t[:, i, :], in_=wqT_d[ts(i, 128), :])
            for i in range(8):
                nc.sync.dma_start(out=wv_t[:, i, :], in_=wvT_d[ts(i, 128), :])
            nc.sync.dma_start(out=m0_t[:], in_=m0_d[:])
            make_identity(nc, ident_t)
            nc.vector.memset(eps_t, LN_EPS)
            nc.vector.memset(zero64_t, 0.0)

            # ---- K-projection of block 0 (PE starts as soon as wk+kt0 land)
            ktproj0_t = kproj_pool.tile([128, 8, BLK], f32r, tag="ktproj_t")
            for o in range(8):
                ps = kps_pool.tile([128, 512], f32, tag="mm")
                for i in range(8):
                    nc.tensor.matmul(
                        ps,
                        wk_t[:, i, ts(o, 128)],
                        kt0_t[:, i, :],
                        start=(i == 0),
                        stop=(i == 7),
                    )
                nc.vector.tensor_copy(ktproj0_t[:, o, :], ps)

            # ---- Q projection: qproj[(h,d) chunk h][d, s] ----
            for o in range(8):
                qps = kps_pool.tile([128, 512], f32, tag="mm")
                for i in range(8):
                    nc.tensor.matmul(
                        qps[:, 0:S],
                        wq_t[:, i, ts(o, 128)],
                        qt_t[:, i, :],
                        start=(i == 0),
                        stop=(i == 7),
                    )
                nc.vector.tensor_copy(qproj_t[:, o, :], qps[:, 0:S])

            # ---- persistent AV accumulator: [d, h, s] (one PSUM bank) ----
            av_ps = avps_pool.tile([128, H, S], f32)

            for b in range(NBLK):
                if b == 0:
                    kt_t = kt0_t
                    biasF_t = biasF0_t
                    ktproj_t = ktproj0_t
                else:
                    # load k^T block
                    kt_t = ktp_pool.tile([128, 8, BLK], bf16, tag="kt_t")
                    for i in range(8):
                        nc.sync.dma_start(
                            out=kt_t[:, i, :], in_=kt_d[ts(i, 128), ts(b, BLK)]
                        )
                    biasF_t = bf_pool.tile([SPB * H, BLK], f32, tag="biasF_t")
                    nc.sync.dma_start(out=biasF_t, in_=biasF_d[b])

                    # K-projection (transposed out): ktp[(h,d) chunk][d, rows]
                    ktproj_t = kproj_pool.tile([128, 8, BLK], f32r, tag="ktproj_t")
                    for o in range(8):
                        ps = kps_pool.tile([128, 512], f32, tag="mm")
                        for i in range(8):
                            nc.tensor.matmul(
                                ps,
                                wk_t[:, i, ts(o, 128)],
                                kt_t[:, i, :],
                                start=(i == 0),
                                stop=(i == 7),
                            )
                        nc.vector.tensor_copy(ktproj_t[:, o, :], ps)

                # V-projection (normal out, bf16): v[rowchunk][row, (h,d)]
                v_t = v_pool.tile([128, RC, DM], bf16)
                for rc in range(RC):
                    for half in range(2):
                        ps = vps_pool.tile([128, 512], f32, tag="vmm")
                        for i in range(8):
                            nc.tensor.matmul(
                                ps,
                                kt_t[:, i, ts(rc, 128)],
                                wv_t[:, i, ts(half, 512)],
                                start=(i == 0),
                                stop=(i == 7),
                            )
                        nc.vector.tensor_copy(v_t[:, rc, ts(half, 512)], ps)

                # QK via block-diagonal Q: out[(s,h), rows]
                qk_ps = qkps_pool.tile([SPB * H, BLK], f32)
                for h in range(H):
                    bdq = bdq_pool.tile([128, SPB * H], f32r)
                    bdq_r = bdq.rearrange("p (s h) -> p s h", h=H)
                    nc.vector.tensor_copy(bdq, zero64_t)
                    nc.vector.tensor_copy(bdq_r[:, :, h], qproj_t[:, h, ts(b, SPB)])
                    nc.tensor.matmul(
                        qk_ps,
                        bdq,
                        ktproj_t[:, h, :],
                        start=(h == 0),
                        stop=(h == 7),
                    )

                # mask/bias over the FULL row (off-diagonal cols get -1e10,
                # so softmax over 512 cols == per-source softmax over 64)
                smf = sm_pool.tile([SPB * H, BLK], f32, tag="smf")
                nc.vector.tensor_add(smf, qk_ps, biasF_t)
                mx = st_pool.tile([SPB * H, 1], f32)
                nc.vector.reduce_max(mx, smf, axis=AX.X)
                negmx = st_pool.tile([SPB * H, 1], f32)
                nc.vector.tensor_scalar_mul(negmx, mx, -1.0)
                ssum = st_pool.tile([SPB * H, 1], f32)
                nc.scalar.activation(
                    smf, smf, AF.Exp, bias=negmx, scale=1.0, accum_out=ssum
                )
                rinv = st_pool.tile([SPB * H, 1], f32)
                nc.vector.reciprocal(rinv, ssum)
                nc.vector.tensor_scalar_mul(smf, smf, rinv)
                # attn output: 8 diagonal-block DMAs (DMA has no partition
                # alignment restriction)
                for sl in range(SPB):
                    nc.sync.dma_start(
                        out=attn_d[b * SPB * H + sl * H : b * SPB * H + (sl + 1) * H, :],
                        in_=smf[sl * H : (sl + 1) * H, ts(sl, NBR)],
                    )
                # transpose prob chunks -> [(s',n), (s,h)]; gather diag cols
                a_t = a_pool.tile([128, RC, H], f32, tag="a")
                for rc in range(RC):
                    pT = tpps_pool.tile([128, SPB * H], f32, tag="tp")
                    nc.tensor.transpose(
                        pT, smf[:, ts(rc, 128)], ident_t[0 : SPB * H, 0 : SPB * H]
                    )
                    nc.vector.tensor_copy(a_t[0:64, rc, :], pT[0:64, ts(2 * rc, H)])
                    nc.vector.tensor_copy(
                        a_t[64:128, rc, :], pT[64:128, ts(2 * rc + 1, H)]
                    )

                # BD probs + AV matmuls accumulating into av_ps
                for h in range(H):
                    bd = bd_pool.tile([128, RC, NBR], bf16)
                    for c in range(RC):
                        nc.vector.tensor_scalar_mul(
                            bd[:, c, :], m0_t[:, b * RC + c, :], a_t[:, c, h : h + 1]
                        )
                    for c in range(RC):
                        nc.tensor.matmul(
                            av_ps[:, h, :],
                            v_t[:, c, ts(h, DK)],
                            bd[:, c, :],
                            start=(b == 0 and h == 0 and c == 0),
                            stop=(b == NBLK - 1 and c == RC - 1),
                        )

            # ---- tail consts (emitted late so startup DMA stays lean) ----
            for i in range(8):
                nc.sync.dma_start(out=fc_t[:, i, :], in_=fcT_d[ts(i, 128), :])
            for i in range(10):
                nc.sync.dma_start(out=fc1_t[:, i, :], in_=fc1T_d[ts(i, 128), :])
            for i in range(2):
                nc.sync.dma_start(out=fc2_t[:, i, :], in_=fc2T_d[ts(i, 128), :])
                nc.sync.dma_start(out=xcatT_t[:, 8 + i, :], in_=srcT_d[ts(i, 128), :])
            nc.sync.dma_start(out=qpfcb_t[:], in_=qpfcb_d[:])
            nc.sync.dma_start(out=fc1b_t[:], in_=fc1b_d[:])
            nc.sync.dma_start(out=fc2b_t[:], in_=fc2b_d[:])

            # ---- tail: fc + residual + LN + merge MLP ----
            av_sb = tail.tile([128, H, S], f32r)
            nc.vector.tensor_copy(av_sb, av_ps)

            x0 = tail.tile([S, DM], f32)
            for half in range(2):
                fps = kps_pool.tile([128, 512], f32, tag="mm")
                for hd in range(8):
                    nc.tensor.matmul(
                        fps[0:S, :],
                        av_sb[:, hd, :],
                        fc_t[:, hd, ts(half, 512)],
                        start=(hd == 0),
                        stop=(hd == 7),
                    )
                nc.vector.tensor_add(
                    x0[:, ts(half, 512)], fps[0:S, :], qpfcb_t[:, ts(half, 512)]
                )

            # LayerNorm over the 1024 free dim
            x0_r = x0.rearrange("p (a b) -> p a b", b=512)
            stats = tail.tile([S, 2, nc.vector.BN_STATS_DIM], f32)
            for sub in range(2):
                nc.vector.bn_stats(stats[:, sub, :], x0_r[:, sub, :])
            mv = tail.tile([S, nc.vector.BN_AGGR_DIM], f32)
            nc.vector.bn_aggr(mv, stats)
            sd = tail.tile([S, 1], f32)
            nc.scalar.activation(sd, mv[:, 1:2], AF.Sqrt, bias=eps_t, scale=1.0)
            rstd = tail.tile([S, 1], f32)
            nc.vector.reciprocal(rstd, sd)
            xn = x0
            nc.vector.tensor_scalar(
                out=xn,
                in0=x0,
                scalar1=mv[:, 0:1],
                scalar2=rstd,
                op0=mybir.AluOpType.subtract,
                op1=mybir.AluOpType.mult,
            )

            # transpose xn into xcatT chunks 0..7 (src already in 8..9)
            for c in range(8):
                tp = tpps_pool.tile([128, S], f32, tag="tp")
                nc.tensor.transpose(tp, xn[:, ts(c, 128)], ident_t[0:S, 0:S])
                nc.vector.tensor_copy(xcatT_t[:, c, :], tp)

            # fc1 + relu
            h1ps = tpps_pool.tile([S, FEAT], f32, tag="tp")
            for c in range(10):
                nc.tensor.matmul(
                    h1ps,
                    xcatT_t[:, c, :],
                    fc1_t[:, c, :],
                    start=(c == 0),
                    stop=(c == 9),
                )
            h1 = tail.tile([S, FEAT], f32)
            nc.vector.tensor_add(h1, h1ps, fc1b_t)
            nc.scalar.activation(h1, h1, AF.Relu)

            # fc2
            h1T = tail.tile([128, 2, S], f32r)
            for c in range(2):
                tp = tpps_pool.tile([128, S], f32, tag="tp")
                nc.tensor.transpose(tp, h1[:, ts(c, 128)], ident_t[0:S, 0:S])
                nc.vector.tensor_copy(h1T[:, c, :], tp)
            zps = tpps_pool.tile([S, FEAT], f32, tag="tp")
            for c in range(2):
                nc.tensor.matmul(
                    zps,
                    h1T[:, c, :],
                    fc2_t[:, c, :],
                    start=(c == 0),
                    stop=(c == 1),
                )
            z_sb = tail.tile([S, FEAT], f32)
            nc.vector.tensor_add(z_sb, zps, fc2b_t)
            nc.sync.dma_start(out=z_d[:], in_=z_sb)

    _fix_multiwait(nc)
    return nc


def _host_prep(inp):
    """Build per-core input dicts from full inputs (host numpy only)."""
    f = np.float32
    src = np.asarray(inp["src"], f)
    src_t = np.asarray(inp["src_t"], f)
    src_p = np.asarray(inp["src_p"], f)
    seq = np.asarray(inp["seq"], f)
    seq_t = np.asarray(inp["seq_t"], f)
    seq_e = np.asarray(inp["seq_e"], f)
    seq_p = np.asarray(inp["seq_p"], f)
    mask = np.asarray(inp["mask"])
    fc_b = np.asarray(inp["fc_b"], f)

    k = np.concatenate([seq, seq_e, seq_t, seq_p], axis=2)          # [B,NGH,DM]
    q = np.concatenate([src, np.zeros_like(src), src_t, src_p], axis=2)

    wqT = np.ascontiguousarray(np.asarray(inp["w_qs"], f).T.astype(ml_dtypes.bfloat16))
    wkT = np.ascontiguousarray(np.asarray(inp["w_ks"], f).T.astype(ml_dtypes.bfloat16))
    wvT = np.ascontiguousarray(np.asarray(inp["w_vs"], f).T.astype(ml_dtypes.bfloat16))
    fcT = np.ascontiguousarray(np.asarray(inp["fc_w"], f).T)
    ln_g = np.asarray(inp["ln_g"], f)
    ln_b = np.asarray(inp["ln_b"], f)
    fc1_w = np.asarray(inp["fc1_w"], f)
    # fold LayerNorm affine into fc1: h = fc1_w @ concat(xn*g+b, src) + b1
    fc1_w_mod = fc1_w.copy()
    fc1_w_mod[:, :DM] = fc1_w[:, :DM] * ln_g[None, :]
    fc1_b_mod = np.asarray(inp["fc1_b"], f) + fc1_w[:, :DM] @ ln_b
    fc1T = np.ascontiguousarray(fc1_w_mod.T)
    fc2T = np.ascontiguousarray(np.asarray(inp["fc2_w"], f).T)
    fc1b = np.ascontiguousarray(np.broadcast_to(fc1_b_mod, (S, FEAT)))
    fc2b = np.ascontiguousarray(np.broadcast_to(np.asarray(inp["fc2_b"], f), (S, FEAT)))

    m0 = np.zeros((128, 32, 64), ml_dtypes.bfloat16)
    for par in range(2):
        for c in range(32):
            m0[par * 64 : (par + 1) * 64, c, 2 * c + par] = 1.0

    scale = np.float32(DK ** -0.5)
    in_maps = []
    for core in range(NCORES):
        b0 = BPC * core
        k_c = k[b0 : b0 + BPC].reshape(ROWS, DM)
        q_c = q[b0 : b0 + BPC].reshape(S, DM)
        maskb = np.where(
            mask[b0 : b0 + BPC].reshape(S, NBR), f(-1e10), f(0.0)
        ).astype(f)
        bf = np.full((NBLK, SPB * H, BLK), f(-1e10), f)
        mb = maskb.reshape(NBLK, SPB, NBR)
        for sl in range(SPB):
            bf[:, sl * H : (sl + 1) * H, sl * NBR : (sl + 1) * NBR] = mb[
                :, sl, None, :
            ]
        in_maps.append(
            {
                "kt": np.ascontiguousarray(k_c.T.astype(ml_dtypes.bfloat16)),
                "qt": np.ascontiguousarray((q_c * scale).T.astype(ml_dtypes.bfloat16)),
                "qpfcb": np.ascontiguousarray(q_c + fc_b[None, :]),
                "srcT": np.ascontiguousarray(
                    src[b0 : b0 + BPC].reshape(S, FEAT).T
                ),
                "biasF": bf,
                "m0": m0,
                "wqT": wqT,
                "wkT": wkT,
                "wvT": wvT,
                "fcT": fcT,
                "fc1T": fc1T,
                "fc2T": fc2T,
                "fc1b": fc1b,
                "fc2b": fc2b,
            }
        )
    return in_maps


def _install_trace_hook():
    """Register the NTFF profile hook (missing antenv.axon_hooks in image)."""
    import sys
    import types

    if "antenv.axon_hooks" in sys.modules:
        return
    import antenv

    mod = types.ModuleType("antenv.axon_hooks")
    _hook = [None]
    mod.set_axon_ntff_profile_hook = lambda h: _hook.__setitem__(0, h)
    mod.get_axon_ntff_profile_hook = lambda: _hook[0]
    sys.modules["antenv.axon_hooks"] = mod
    antenv.axon_hooks = mod
    try:
        from trn_agent_boot.trn_boot import _ntff_profile_via_ctypes

        h = _ntff_profile_via_ctypes("/opt/axon/libaxon_pjrt.so")
        if h is not None:
            mod.set_axon_ntff_profile_hook(h)
    except Exception:
        pass


def kernel(**inputs):
    global LAST_EXEC_NS
    from concourse.bass_utils import run_bass_kernel_spmd

    trace = bool(os.environ.get("BASS_KERNEL_TRACE"))
    if trace:
        _install_trace_hook()

    if "nc" not in _CACHE:
        _CACHE["nc"] = _build_nc()
    nc = _CACHE["nc"]

    in_maps = _host_prep(inputs)
    kwargs = {}
    if trace:
        kwargs["trace"] = True
        td = os.environ.get("BASS_KERNEL_TRACE_DIR")
        if td:
            os.makedirs(td, exist_ok=True)
            kwargs["tmpdir"] = td
    res = run_bass_kernel_spmd(nc, in_maps, list(range(NCORES)), **kwargs)
    LAST_EXEC_NS = res.exec_time_ns

    z = np.stack([res.results[i]["z"] for i in range(NCORES)]).reshape(B, NS, FEAT)
    attn = np.stack([res.results[i]["attn"] for i in range(NCORES)]).reshape(
        B, NS, H, NBR
    )
    return z, attn


# revision 28
# speedup vs baseline: 1.2320x; 1.0414x over previous
"""Trainium2 Bass kernel for the TGAT-style AttnModel (gnn_message_passing).

Contract: kernel(**inputs) takes FULL unsharded numpy inputs (as produced by
setup_inputs()) and returns the FULL output tuple (z, attn).

Strategy: pure data parallel over batch B=16 -> 2 batches per NeuronCore
(8 cores). Per core: 64 sources, 4096 neighbor rows. All projections run as
fp32r matmuls on the PE; attention uses a block-diagonal-Q trick for QK and a
block-diagonal-probs (BD) trick for attn@V; softmax in fp32 on DVE/ACT.
Host-side prep does the concats/transposes (pure data movement).
"""

import os
import numpy as np

import ml_dtypes

# ---- model constants (hardcoded; kernel.py must be self-contained) ----
B, NS, NGH, NBR = 16, 32, 2048, 64
FEAT = 256
DM, H, DK = 1024, 8, 128
LN_EPS = 1e-5

NCORES = 8
BPC = B // NCORES          # 2 batches per core
S = BPC * NS               # 64 sources per core
ROWS = BPC * NGH           # 4096 k-rows per core
NBLK = 8
BLK = ROWS // NBLK         # 512 rows per block
SPB = S // NBLK            # 8 sources per block
RC = BLK // 128            # 4 row-chunks of 128 per block

_CACHE = {}
LAST_EXEC_NS = None


def _patch_drain():
    """walrus in this container rejects instructions with >1 sync wait on the
    Drain ctrl struct; split the TileContext tail-drain waits into one drain
    per semaphore."""
    import concourse.tile as tile
    from concourse import mybir
    from concourse.vector_clock import ScopedClock

    if getattr(tile.TileContext, "_drain_patched", False):
        return

    def _drain_and_barrier(self, tick_clock, wait_clock):
        nc = self.nc
        drain_inst = nc.sync.drain()
        wait_clock.add_sem_waits(
            drain_inst.ins, ScopedClock({None: tick_clock.global_clock})
        )
        ri = drain_inst.ins
        waits = list(ri.sync_info.on_wait)
        ri.sync_info = mybir.SyncInfo(on_wait=waits[:1], on_update=[])
        for w in waits[1:]:
            d2 = nc.sync.drain()
            d2.ins.sync_info = mybir.SyncInfo(on_wait=[w], on_update=[])
        nc.all_engine_barrier()
        popped = nc._tile_sem_poison_stack.pop()
        assert popped is self._sem_poison
        nc.clear_and_free_semaphores(list(self.sems.allocated().values()))
        nc.all_engine_barrier()

    tile.TileContext._drain_and_barrier = _drain_and_barrier
    tile.TileContext._drain_patched = True


def _fix_multiwait(nc):
    """walrus in this container accepts very few sync commands per
    instruction (1 wait on NOP/Drain ctrl; a wait+update on engine ctrl).
    Conservatively rewrite every block so each instruction carries at most
    ONE wait: excess waits move to same-engine NoOp carriers inserted
    immediately before the instruction — identical semantics (same engine,
    same program position), so no scheduling or deadlock risk."""
    from concourse import mybir

    ctr = [0]
    for f in nc.m.functions:
        for bb in f.blocks:
            out = []
            changed = False
            for inst in bb.instructions:
                si = inst.sync_info
                if si is not None and len(si.on_wait) > 1:
                    waits = list(si.on_wait)
                    for w in waits[:-1]:
                        ctr[0] += 1
                        nop = mybir.InstNoOp(
                            name=f"I-wfix-{ctr[0]}", ins=[], outs=[]
                        )
                        nop.engine = inst.engine
                        nop.sync_info = mybir.SyncInfo(
                            on_wait=[w], on_update=[]
                        )
                        out.append(nop)
                    inst.sync_info = mybir.SyncInfo(
                        on_wait=[waits[-1]], on_update=list(si.on_update)
                    )
                    changed = True
                out.append(inst)
            if changed:
                bb.instructions = out


def _build_nc():
    import concourse.bass as bass
    import concourse.tile as tile
    import concourse.tile_utils as tile_utils
    from concourse import mybir
    from concourse.bass import ts
    from concourse.masks import make_identity

    _patch_drain()
    # stale 192KB cap; cayman has 208KB usable per partition
    tile_utils.max_sbuf_usage = 207 * 1024

    f32 = mybir.dt.float32
    f32r = mybir.dt.float32r
    bf16 = mybir.dt.bfloat16
    AX = mybir.AxisListType
    AF = mybir.ActivationFunctionType

    nc = bass.Bass(target_bir_lowering=False)

    # ---- DRAM parameters (per-core views, host-prepared) ----
    kt_d = nc.declare_dram_parameter("kt", [DM, ROWS], bf16, isOutput=False)
    qt_d = nc.declare_dram_parameter("qt", [DM, S], bf16, isOutput=False)
    qpfcb_d = nc.declare_dram_parameter("qpfcb", [S, DM], f32, isOutput=False)
    srcT_d = nc.declare_dram_parameter("srcT", [FEAT, S], f32r, isOutput=False)
    biasF_d = nc.declare_dram_parameter("biasF", [NBLK, SPB * H, BLK], f32, isOutput=False)
    m0_d = nc.declare_dram_parameter("m0", [128, 32, 64], bf16, isOutput=False)
    wqT_d = nc.declare_dram_parameter("wqT", [DM, DM], bf16, isOutput=False)
    wkT_d = nc.declare_dram_parameter("wkT", [DM, DM], bf16, isOutput=False)
    wvT_d = nc.declare_dram_parameter("wvT", [DM, DM], bf16, isOutput=False)
    fcT_d = nc.declare_dram_parameter("fcT", [DM, DM], f32r, isOutput=False)
    fc1T_d = nc.declare_dram_parameter("fc1T", [DM + FEAT, FEAT], f32r, isOutput=False)
    fc2T_d = nc.declare_dram_parameter("fc2T", [FEAT, FEAT], f32r, isOutput=False)
    fc1b_d = nc.declare_dram_parameter("fc1b", [S, FEAT], f32, isOutput=False)
    fc2b_d = nc.declare_dram_parameter("fc2b", [S, FEAT], f32, isOutput=False)
    z_d = nc.declare_dram_parameter("z", [S, FEAT], f32, isOutput=True)
    attn_d = nc.declare_dram_parameter("attn", [S * H, NBR], f32, isOutput=True)

    with tile.TileContext(nc) as tc:
        with (
            tc.tile_pool(name="const", bufs=1) as const,
            tc.tile_pool(name="kt", bufs=2) as ktp_pool,
            tc.tile_pool(name="kproj", bufs=2) as kproj_pool,
            tc.tile_pool(name="v", bufs=2) as v_pool,
            tc.tile_pool(name="bdq", bufs=2) as bdq_pool,
            tc.tile_pool(name="sm", bufs=2) as sm_pool,
            tc.tile_pool(name="bf", bufs=2) as bf_pool,
            tc.tile_pool(name="st", bufs=8) as st_pool,
            tc.tile_pool(name="a", bufs=2) as a_pool,
            tc.tile_pool(name="bd", bufs=2) as bd_pool,
            tc.tile_pool(name="tail", bufs=1) as tail,
            tc.tile_pool(name="kps", bufs=2, space="PSUM") as kps_pool,
            tc.tile_pool(name="vps", bufs=2, space="PSUM") as vps_pool,
            tc.tile_pool(name="qkps", bufs=1, space="PSUM") as qkps_pool,
            tc.tile_pool(name="avps", bufs=1, space="PSUM") as avps_pool,
            tc.tile_pool(name="tpps", bufs=2, space="PSUM") as tpps_pool,
        ):
            # ---- constants / weights ----
            wk_t = const.tile([128, 8, DM], bf16)
            wv_t = const.tile([128, 8, DM], bf16)
            fc_t = const.tile([128, 8, DM], f32r)  # holds wqT first, fcT later
            fc1_t = const.tile([128, 10, FEAT], f32r)
            fc2_t = const.tile([128, 2, FEAT], f32r)
            m0_t = const.tile([128, 32, 64], bf16)
            qpfcb_t = const.tile([S, DM], f32)
            fc1b_t = const.tile([S, FEAT], f32)
            fc2b_t = const.tile([S, FEAT], f32)
            xcatT_t = const.tile([128, 10, S], f32r)
            qproj_t = const.tile([128, 8, S], f32)
            ident_t = const.tile([128, 128], f32)
            eps_t = const.tile([S, 1], f32)
            zero64_t = const.tile([128, SPB * H], f32)

            # startup-ordered loads: wk -> kt(b0)+biasF(b0) -> qt+wqT -> wv -> m0
            for i in range(8):
                nc.sync.dma_start(out=wk_t[:, i, :], in_=wkT_d[ts(i, 128), :])
            kt0_t = ktp_pool.tile([128, 8, BLK], bf16, tag="kt_t")
            for i in range(8):
                nc.sync.dma_start(out=kt0_t[:, i, :], in_=kt_d[ts(i, 128), ts(0, BLK)])
            biasF0_t = bf_pool.tile([SPB * H, BLK], f32, tag="biasF_t")
            nc.sync.dma_start(out=biasF0_t, in_=biasF_d[0])
            qt_t = const.tile([128, 8, S], bf16)
            wq_t = const.tile([128, 8, DM], bf16)
            for i in range(8):
                nc.sync.dma_start(out=qt_t[:, i, :], in_=qt_d[ts(i, 128), :])
                nc.sync.dma_start(out=wq_t[:, i, :], in_=wqT_d[ts(i, 128), :])
            for i in range(8):
                nc.sync.dma_start(out=wv_t[:, i, :], in_=wvT_d[ts(i, 128), :])
            nc.sync.dma_start(out=m0_t[:], in_=m0_d[:])
            make_identity(nc, ident_t)
            nc.vector.memset(eps_t, LN_EPS)
            nc.vector.memset(zero64_t, 0.0)

            # ---- K-projection of block 0 (PE starts as soon as wk+kt0 land)
            ktproj0_t = kproj_pool.tile([128, 8, BLK], f32r, tag="ktproj_t")
            for o in range(8):
                ps = kps_pool.tile([128, 512], f32, tag="mm")
                for i in range(8):
                    nc.tensor.matmul(
                        ps,
                        wk_t[:, i, ts(o, 128)],
                        kt0_t[:, i, :],
                        start=(i == 0),
                        stop=(i == 7),
                    )
                nc.vector.tensor_copy(ktproj0_t[:, o, :], ps)

            # ---- Q projection: qproj[(h,d) chunk h][d, s] ----
            for o in range(8):
                qps = kps_pool.tile([128, 512], f32, tag="mm")
                for i in range(8):
                    nc.tensor.matmul(
                        qps[:, 0:S],
                        wq_t[:, i, ts(o, 128)],
                        qt_t[:, i, :],
                        start=(i == 0),
                        stop=(i == 7),
                    )
                nc.vector.tensor_copy(qproj_t[:, o, :], qps[:, 0:S])

            # ---- persistent AV accumulator: [d, h, s] (one PSUM bank) ----
            av_ps = avps_pool.tile([128, H, S], f32)

            for b in range(NBLK):
                if b == 0:
                    kt_t = kt0_t
                    biasF_t = biasF0_t
                    ktproj_t = ktproj0_t
                else:
                    # load k^T block
                    kt_t = ktp_pool.tile([128, 8, BLK], bf16, tag="kt_t")
                    for i in range(8):
                        nc.sync.dma_start(
                            out=kt_t[:, i, :], in_=kt_d[ts(i, 128), ts(b, BLK)]
                        )
                    biasF_t = bf_pool.tile([SPB * H, BLK], f32, tag="biasF_t")
                    nc.sync.dma_start(out=biasF_t, in_=biasF_d[b])

                    # K-projection (transposed out): ktp[(h,d) chunk][d, rows]
                    ktproj_t = kproj_pool.tile([128, 8, BLK], f32r, tag="ktproj_t")
                    for o in range(8):
                        ps = kps_pool.tile([128, 512], f32, tag="mm")
                        for i in range(8):
                            nc.tensor.matmul(
                                ps,
                                wk_t[:, i, ts(o, 128)],
                                kt_t[:, i, :],
                                start=(i == 0),
                                stop=(i == 7),
                            )
                        nc.vector.tensor_copy(ktproj_t[:, o, :], ps)

                # V-projection (normal out, bf16): v[rowchunk][row, (h,d)]
                v_t = v_pool.tile([128, RC, DM], bf16)
                for rc in range(RC):
                    for half in range(2):
                        ps = vps_pool.tile([128, 512], f32, tag="vmm")
                        for i in range(8):
                            nc.tensor.matmul(
                                ps,
                                kt_t[:, i, ts(rc, 128)],
                                wv_t[:, i, ts(half, 512)],
                                start=(i == 0),
                                stop=(i == 7),
                            )
                        if rc % 2 == 0:
                            nc.vector.tensor_copy(v_t[:, rc, ts(half, 512)], ps)
                        else:
                            nc.scalar.copy(v_t[:, rc, ts(half, 512)], ps)

                # QK via block-diagonal Q: out[(s,h), rows]
                qk_ps = qkps_pool.tile([SPB * H, BLK], f32)
                for h in range(H):
                    bdq = bdq_pool.tile([128, SPB * H], f32r)
                    bdq_r = bdq.rearrange("p (s h) -> p s h", h=H)
                    nc.vector.tensor_copy(bdq, zero64_t)
                    nc.vector.tensor_copy(bdq_r[:, :, h], qproj_t[:, h, ts(b, SPB)])
                    nc.tensor.matmul(
                        qk_ps,
                        bdq,
                        ktproj_t[:, h, :],
                        start=(h == 0),
                        stop=(h == 7),
                    )

                # mask/bias over the FULL row (off-diagonal cols get -1e10,
                # so softmax over 512 cols == per-source softmax over 64)
                smf = sm_pool.tile([SPB * H, BLK], f32, tag="smf")
                nc.vector.tensor_add(smf, qk_ps, biasF_t)
                mx = st_pool.tile([SPB * H, 1], f32)
                nc.vector.reduce_max(mx, smf, axis=AX.X)
                negmx = st_pool.tile([SPB * H, 1], f32)
                nc.vector.tensor_scalar_mul(negmx, mx, -1.0)
                ssum = st_pool.tile([SPB * H, 1], f32)
                nc.scalar.activation(
                    smf, smf, AF.Exp, bias=negmx, scale=1.0, accum_out=ssum
                )
                rinv = st_pool.tile([SPB * H, 1], f32)
                nc.vector.reciprocal(rinv, ssum)
                nc.vector.tensor_scalar_mul(smf, smf, rinv)
                # attn output: 8 diagonal-block DMAs (DMA has no partition
                # alignment restriction)
                for sl in range(SPB):
                    nc.sync.dma_start(
                        out=attn_d[b * SPB * H + sl * H : b * SPB * H + (sl + 1) * H, :],
                        in_=smf[sl * H : (sl + 1) * H, ts(sl, NBR)],
                    )
                # transpose prob chunks -> [(s',n), (s,h)]; gather diag cols
                a_t = a_pool.tile([128, RC, H], f32, tag="a")
                for rc in range(RC):
                    pT = tpps_pool.tile([128, SPB * H], f32, tag="tp")
                    nc.tensor.transpose(
                        pT, smf[:, ts(rc, 128)], ident_t[0 : SPB * H, 0 : SPB * H]
                    )
                    nc.vector.tensor_copy(a_t[0:64, rc, :], pT[0:64, ts(2 * rc, H)])
                    nc.vector.tensor_copy(
                        a_t[64:128, rc, :], pT[64:128, ts(2 * rc + 1, H)]
                    )

                # BD probs + AV matmuls accumulating into av_ps
                for h in range(H):
                    bd = bd_pool.tile([128, RC, NBR], bf16)
                    a_col = a_t[:, :, h]
                    a_bc = bass.AP(
                        tensor=a_col.tensor,
                        offset=a_col.offset,
                        ap=list(a_col.ap) + [[0, NBR]],
                    )
                    nc.vector.tensor_mul(
                        bd, m0_t[:, b * RC : (b + 1) * RC, :], a_bc
                    )
                    for c in range(RC):
                        nc.tensor.matmul(
                            av_ps[:, h, :],
                            v_t[:, c, ts(h, DK)],
                            bd[:, c, :],
                            start=(b == 0 and h == 0 and c == 0),
                            stop=(b == NBLK - 1 and c == RC - 1),
                        )

            # ---- tail consts (emitted late so startup DMA stays lean) ----
            for i in range(8):
                nc.sync.dma_start(out=fc_t[:, i, :], in_=fcT_d[ts(i, 128), :])
            for i in range(10):
                nc.sync.dma_start(out=fc1_t[:, i, :], in_=fc1T_d[ts(i, 128), :])
            for i in range(2):
                nc.sync.dma_start(out=fc2_t[:, i, :], in_=fc2T_d[ts(i, 128), :])
                nc.sync.dma_start(out=xcatT_t[:, 8 + i, :], in_=srcT_d[ts(i, 128), :])
            nc.sync.dma_start(out=qpfcb_t[:], in_=qpfcb_d[:])
            nc.sync.dma_start(out=fc1b_t[:], in_=fc1b_d[:])
            nc.sync.dma_start(out=fc2b_t[:], in_=fc2b_d[:])

            # ---- tail: fc + residual + LN + merge MLP ----
            av_sb = tail.tile([128, H, S], f32r)
            nc.vector.tensor_copy(av_sb, av_ps)

            x0 = tail.tile([S, DM], f32)
            for half in range(2):
                fps = kps_pool.tile([128, 512], f32, tag="mm")
                for hd in range(8):
                    nc.tensor.matmul(
                        fps[0:S, :],
                        av_sb[:, hd, :],
                        fc_t[:, hd, ts(half, 512)],
                        start=(hd == 0),
                        stop=(hd == 7),
                    )
                nc.vector.tensor_add(
                    x0[:, ts(half, 512)], fps[0:S, :], qpfcb_t[:, ts(half, 512)]
                )

            # LayerNorm over the 1024 free dim
            x0_r = x0.rearrange("p (a b) -> p a b", b=512)
            stats = tail.tile([S, 2, nc.vector.BN_STATS_DIM], f32)
            for sub in range(2):
                nc.vector.bn_stats(stats[:, sub, :], x0_r[:, sub, :])
            mv = tail.tile([S, nc.vector.BN_AGGR_DIM], f32)
            nc.vector.bn_aggr(mv, stats)
            sd = tail.tile([S, 1], f32)
            nc.scalar.activation(sd, mv[:, 1:2], AF.Sqrt, bias=eps_t, scale=1.0)
            rstd = tail.tile([S, 1], f32)
            nc.vector.reciprocal(rstd, sd)
            xn = x0
            nc.vector.tensor_scalar(
                out=xn,
                in0=x0,
                scalar1=mv[:, 0:1],
                scalar2=rstd,
                op0=mybir.AluOpType.subtract,
                op1=mybir.AluOpType.mult,
            )

            # transpose xn into xcatT chunks 0..7 (src already in 8..9)
            for c in range(8):
                tp = tpps_pool.tile([128, S], f32, tag="tp")
                nc.tensor.transpose(tp, xn[:, ts(c, 128)], ident_t[0:S, 0:S])
                nc.vector.tensor_copy(xcatT_t[:, c, :], tp)

            # fc1 + relu
            h1ps = tpps_pool.tile([S, FEAT], f32, tag="tp")
            for c in range(10):
                nc.tensor.matmul(
                    h1ps,
                    xcatT_t[:, c, :],
                    fc1_t[:, c, :],
                    start=(c == 0),
                    stop=(c == 9),
                )
            h1 = tail.tile([S, FEAT], f32)
            nc.vector.tensor_add(h1, h1ps, fc1b_t)
            nc.scalar.activation(h1, h1, AF.Relu)

            # fc2
            h1T = tail.tile([128, 2, S], f32r)
            for c in range(2):
                tp = tpps_pool.tile([128, S], f32, tag="tp")
                nc.tensor.transpose(tp, h1[:, ts(c, 128)], ident_t[0:S, 0:S])
                nc.vector.tensor_copy(h1T[:, c, :], tp)
            zps = tpps_pool.tile([S, FEAT], f32, tag="tp")
            for c in range(2):
                nc.tensor.matmul(
                    zps,
                    h1T[:, c, :],
                    fc2_t[:, c, :],
                    start=(c == 0),
                    stop=(c == 1),
                )
            z_sb = tail.tile([S, FEAT], f32)
            nc.vector.tensor_add(z_sb, zps, fc2b_t)
            nc.sync.dma_start(out=z_d[:], in_=z_sb)

    _fix_multiwait(nc)
    return nc


def _host_prep(inp):
    """Build per-core input dicts from full inputs (host numpy only)."""
    f = np.float32
    src = np.asarray(inp["src"], f)
    src_t = np.asarray(inp["src_t"], f)
    src_p = np.asarray(inp["src_p"], f)
    seq = np.asarray(inp["seq"], f)
    seq_t = np.asarray(inp["seq_t"], f)
    seq_e = np.asarray(inp["seq_e"], f)
    seq_p = np.asarray(inp["seq_p"], f)
    mask = np.asarray(inp["mask"])
    fc_b = np.asarray(inp["fc_b"], f)

    k = np.concatenate([seq, seq_e, seq_t, seq_p], axis=2)          # [B,NGH,DM]
    q = np.concatenate([src, np.zeros_like(src), src_t, src_p], axis=2)

    wqT = np.ascontiguousarray(np.asarray(inp["w_qs"], f).T.astype(ml_dtypes.bfloat16))
    wkT = np.ascontiguousarray(np.asarray(inp["w_ks"], f).T.astype(ml_dtypes.bfloat16))
    wvT = np.ascontiguousarray(np.asarray(inp["w_vs"], f).T.astype(ml_dtypes.bfloat16))
    fcT = np.ascontiguousarray(np.asarray(inp["fc_w"], f).T)
    ln_g = np.asarray(inp["ln_g"], f)
    ln_b = np.asarray(inp["ln_b"], f)
    fc1_w = np.asarray(inp["fc1_w"], f)
    # fold LayerNorm affine into fc1: h = fc1_w @ concat(xn*g+b, src) + b1
    fc1_w_mod = fc1_w.copy()
    fc1_w_mod[:, :DM] = fc1_w[:, :DM] * ln_g[None, :]
    fc1_b_mod = np.asarray(inp["fc1_b"], f) + fc1_w[:, :DM] @ ln_b
    fc1T = np.ascontiguousarray(fc1_w_mod.T)
    fc2T = np.ascontiguousarray(np.asarray(inp["fc2_w"], f).T)
    fc1b = np.ascontiguousarray(np.broadcast_to(fc1_b_mod, (S, FEAT)))
    fc2b = np.ascontiguousarray(np.broadcast_to(np.asarray(inp["fc2_b"], f), (S, FEAT)))

    m0 = np.zeros((128, 32, 64), ml_dtypes.bfloat16)
    for par in range(2):
        for c in range(32):
            m0[par * 64 : (par + 1) * 64, c, 2 * c + par] = 1.0

    scale = np.float32(DK ** -0.5)
    in_maps = []
    for core in range(NCORES):
        b0 = BPC * core
        k_c = k[b0 : b0 + BPC].reshape(ROWS, DM)
        q_c = q[b0 : b0 + BPC].reshape(S, DM)
        maskb = np.where(
            mask[b0 : b0 + BPC].reshape(S, NBR), f(-1e10), f(0.0)
        ).astype(f)
        bf = np.full((NBLK, SPB * H, BLK), f(-1e10), f)
        mb = maskb.reshape(NBLK, SPB, NBR)
        for sl in range(SPB):
            bf[:, sl * H : (sl + 1) * H, sl * NBR : (sl + 1) * NBR] = mb[
                :, sl, None, :
            ]
        in_maps.append(
            {
                "kt": np.ascontiguousarray(k_c.T.astype(ml_dtypes.bfloat16)),
                "qt": np.ascontiguousarray((q_c * scale).T.astype(ml_dtypes.bfloat16)),
                "qpfcb": np.ascontiguousarray(q_c + fc_b[None, :]),
                "srcT": np.ascontiguousarray(
                    src[b0 : b0 + BPC].reshape(S, FEAT).T
                ),
                "biasF": bf,
                "m0": m0,
                "wqT": wqT,
                "wkT": wkT,
                "wvT": wvT,
                "fcT": fcT,
                "fc1T": fc1T,
                "fc2T": fc2T,
                "fc1b": fc1b,
                "fc2b": fc2b,
            }
        )
    return in_maps


def _install_trace_hook():
    """Register the NTFF profile hook (missing antenv.axon_hooks in image)."""
    import sys
    import types

    if "antenv.axon_hooks" in sys.modules:
        return
    import antenv

    mod = types.ModuleType("antenv.axon_hooks")
    _hook = [None]
    mod.set_axon_ntff_profile_hook = lambda h: _hook.__setitem__(0, h)
    mod.get_axon_ntff_profile_hook = lambda: _hook[0]
    sys.modules["antenv.axon_hooks"] = mod
    antenv.axon_hooks = mod
    try:
        from trn_agent_boot.trn_boot import _ntff_profile_via_ctypes

        h = _ntff_profile_via_ctypes("/opt/axon/libaxon_pjrt.so")
        if h is not None:
            mod.set_axon_ntff_profile_hook(h)
    except Exception:
        pass


def kernel(**inputs):
    global LAST_EXEC_NS
    from concourse.bass_utils import run_bass_kernel_spmd

    trace = bool(os.environ.get("BASS_KERNEL_TRACE"))
    if trace:
        _install_trace_hook()

    if "nc" not in _CACHE:
        _CACHE["nc"] = _build_nc()
    nc = _CACHE["nc"]

    in_maps = _host_prep(inputs)
    kwargs = {}
    if trace:
        kwargs["trace"] = True
        td = os.environ.get("BASS_KERNEL_TRACE_DIR")
        if td:
            os.makedirs(td, exist_ok=True)
            kwargs["tmpdir"] = td
    res = run_bass_kernel_spmd(nc, in_maps, list(range(NCORES)), **kwargs)
    LAST_EXEC_NS = res.exec_time_ns

    z = np.stack([res.results[i]["z"] for i in range(NCORES)]).reshape(B, NS, FEAT)
    attn = np.stack([res.results[i]["attn"] for i in range(NCORES)]).reshape(
        B, NS, H, NBR
    )
    return z, attn


# revision 40
# speedup vs baseline: 1.7327x; 1.4065x over previous
"""Trainium2 Bass kernel for the TGAT-style AttnModel (gnn_message_passing).

Contract: kernel(**inputs) takes FULL unsharded numpy inputs (as produced by
setup_inputs()) and returns the FULL output tuple (z, attn).

Strategy: pure data parallel over batch B=16 -> 2 batches per NeuronCore
(8 cores). Per core: 64 sources, 4096 neighbor rows. All projections run as
fp32r matmuls on the PE; attention uses a block-diagonal-Q trick for QK and a
block-diagonal-probs (BD) trick for attn@V; softmax in fp32 on DVE/ACT.
Host-side prep does the concats/transposes (pure data movement).
"""

import os
import numpy as np

import ml_dtypes

# ---- model constants (hardcoded; kernel.py must be self-contained) ----
B, NS, NGH, NBR = 16, 32, 2048, 64
FEAT = 256
DM, H, DK = 1024, 8, 128
LN_EPS = 1e-5

NCORES = 8
BPC = B // NCORES          # 2 batches per core
S = BPC * NS               # 64 sources per core
ROWS = BPC * NGH           # 4096 k-rows per core
NBLK = 8
BLK = ROWS // NBLK         # 512 rows per block
SPB = S // NBLK            # 8 sources per block
RC = BLK // 128            # 4 row-chunks of 128 per block

_CACHE = {}
LAST_EXEC_NS = None


def _patch_drain():
    """walrus in this container rejects instructions with >1 sync wait on the
    Drain ctrl struct; split the TileContext tail-drain waits into one drain
    per semaphore."""
    import concourse.tile as tile
    from concourse import mybir
    from concourse.vector_clock import ScopedClock

    if getattr(tile.TileContext, "_drain_patched", False):
        return

    def _drain_and_barrier(self, tick_clock, wait_clock):
        nc = self.nc
        drain_inst = nc.sync.drain()
        wait_clock.add_sem_waits(
            drain_inst.ins, ScopedClock({None: tick_clock.global_clock})
        )
        ri = drain_inst.ins
        waits = list(ri.sync_info.on_wait)
        ri.sync_info = mybir.SyncInfo(on_wait=waits[:1], on_update=[])
        for w in waits[1:]:
            d2 = nc.sync.drain()
            d2.ins.sync_info = mybir.SyncInfo(on_wait=[w], on_update=[])
        nc.all_engine_barrier()
        popped = nc._tile_sem_poison_stack.pop()
        assert popped is self._sem_poison
        nc.clear_and_free_semaphores(list(self.sems.allocated().values()))
        nc.all_engine_barrier()

    tile.TileContext._drain_and_barrier = _drain_and_barrier
    tile.TileContext._drain_patched = True


def _fix_multiwait(nc):
    """walrus in this container accepts very few sync commands per
    instruction (1 wait on NOP/Drain ctrl; a wait+update on engine ctrl).
    Conservatively rewrite every block so each instruction carries at most
    ONE wait: excess waits move to same-engine NoOp carriers inserted
    immediately before the instruction — identical semantics (same engine,
    same program position), so no scheduling or deadlock risk."""
    from concourse import mybir

    ctr = [0]
    for f in nc.m.functions:
        for bb in f.blocks:
            out = []
            changed = False
            for inst in bb.instructions:
                si = inst.sync_info
                if si is not None and len(si.on_wait) > 1:
                    waits = list(si.on_wait)
                    for w in waits[:-1]:
                        ctr[0] += 1
                        nop = mybir.InstDrain(
                            name=f"I-wfix-{ctr[0]}",
                            ins=[],
                            outs=[],
                            is_reset_sema=False,
                        )
                        nop.engine = inst.engine
                        nop.sync_info = mybir.SyncInfo(
                            on_wait=[w], on_update=[]
                        )
                        out.append(nop)
                    inst.sync_info = mybir.SyncInfo(
                        on_wait=[waits[-1]], on_update=list(si.on_update)
                    )
                    changed = True
                out.append(inst)
            if changed:
                bb.instructions = out


def _build_nc():
    import concourse.bass as bass
    import concourse.tile as tile
    import concourse.tile_utils as tile_utils
    from concourse import mybir
    from concourse.bass import ts
    from concourse.masks import make_identity

    _patch_drain()
    # stale 192KB cap; cayman has 208KB usable per partition
    tile_utils.max_sbuf_usage = 207 * 1024

    f32 = mybir.dt.float32
    f32r = mybir.dt.float32r
    bf16 = mybir.dt.bfloat16
    AX = mybir.AxisListType
    AF = mybir.ActivationFunctionType

    nc = bass.Bass(target_bir_lowering=False)

    # ---- DRAM parameters (per-core views, host-prepared) ----
    kt_d = nc.declare_dram_parameter("kt", [DM, ROWS], bf16, isOutput=False)
    qt_d = nc.declare_dram_parameter("qt", [DM, S], bf16, isOutput=False)
    qpfcb_d = nc.declare_dram_parameter("qpfcb", [S, DM], f32, isOutput=False)
    srcT_d = nc.declare_dram_parameter("srcT", [FEAT, S], f32r, isOutput=False)
    biasF_d = nc.declare_dram_parameter("biasF", [NBLK, SPB * H, BLK], f32, isOutput=False)
    kn_d = nc.declare_dram_parameter("kn", [ROWS, DM], bf16, isOutput=False)
    m0p_d = nc.declare_dram_parameter("m0p", [128, RC, SPB * H], f32, isOutput=False)
    wqT_d = nc.declare_dram_parameter("wqT", [DM, DM], bf16, isOutput=False)
    wkT_d = nc.declare_dram_parameter("wkT", [DM, DM], bf16, isOutput=False)
    wvT_d = nc.declare_dram_parameter("wvT", [DM, DM], bf16, isOutput=False)
    fcT_d = nc.declare_dram_parameter("fcT", [DM, DM], f32r, isOutput=False)
    fc1T_d = nc.declare_dram_parameter("fc1T", [DM + FEAT, FEAT], f32r, isOutput=False)
    fc2T_d = nc.declare_dram_parameter("fc2T", [FEAT, FEAT], f32r, isOutput=False)
    fc1b_d = nc.declare_dram_parameter("fc1b", [S, FEAT], f32, isOutput=False)
    fc2b_d = nc.declare_dram_parameter("fc2b", [S, FEAT], f32, isOutput=False)
    z_d = nc.declare_dram_parameter("z", [S, FEAT], f32, isOutput=True)
    attn_d = nc.declare_dram_parameter("attn", [S * H, NBR], f32, isOutput=True)

    with tile.TileContext(nc) as tc:
        with (
            tc.tile_pool(name="const", bufs=1) as const,
            tc.tile_pool(name="kt", bufs=2) as ktp_pool,
            tc.tile_pool(name="kproj", bufs=2) as kproj_pool,
            tc.tile_pool(name="kn", bufs=2) as kn_pool,
            tc.tile_pool(name="bdq", bufs=2) as bdq_pool,
            tc.tile_pool(name="sm", bufs=2) as sm_pool,
            tc.tile_pool(name="bf", bufs=2) as bf_pool,
            tc.tile_pool(name="st", bufs=8) as st_pool,
            tc.tile_pool(name="bdall", bufs=2) as bdall_pool,
            tc.tile_pool(name="tail", bufs=1) as tail,
            tc.tile_pool(name="kps", bufs=2, space="PSUM") as kps_pool,
            tc.tile_pool(name="vps", bufs=2, space="PSUM") as vps_pool,
            tc.tile_pool(name="qkps", bufs=1, space="PSUM") as qkps_pool,
            tc.tile_pool(name="avps", bufs=1, space="PSUM") as avps_pool,
            tc.tile_pool(name="tpps", bufs=2, space="PSUM") as tpps_pool,
        ):
            # ---- constants / weights ----
            wk_t = const.tile([128, 8, DM], bf16)
            wv_t = const.tile([128, 8, DM], bf16)
            fc_t = const.tile([128, 8, DM], f32r)  # holds wqT first, fcT later
            fc1_t = const.tile([128, 10, FEAT], f32r)
            fc2_t = const.tile([128, 2, FEAT], f32r)
            m0p_t = const.tile([128, RC, SPB * H], f32)
            ak_sb = const.tile([128, 8, S * H], bf16)
            qpfcb_t = const.tile([S, DM], f32)
            fc1b_t = const.tile([S, FEAT], f32)
            fc2b_t = const.tile([S, FEAT], f32)
            xcatT_t = const.tile([128, 10, S], f32r)
            qproj_t = const.tile([128, 8, S], f32)
            ident_t = const.tile([128, 128], f32)
            eps_t = const.tile([S, 1], f32)
            zero64_t = const.tile([128, SPB * H], f32)

            # startup-ordered loads: wk -> kt(b0)+biasF(b0) -> qt+wqT -> wv -> m0
            for i in range(8):
                nc.sync.dma_start(out=wk_t[:, i, :], in_=wkT_d[ts(i, 128), :])
            kt0_t = ktp_pool.tile([128, 8, BLK], bf16, tag="kt_t")
            for i in range(8):
                nc.sync.dma_start(out=kt0_t[:, i, :], in_=kt_d[ts(i, 128), ts(0, BLK)])
            biasF0_t = bf_pool.tile([SPB * H, BLK], f32, tag="biasF_t")
            nc.sync.dma_start(out=biasF0_t, in_=biasF_d[0])
            qt_t = const.tile([128, 8, S], bf16)
            wq_t = const.tile([128, 8, DM], bf16)
            for i in range(8):
                nc.sync.dma_start(out=qt_t[:, i, :], in_=qt_d[ts(i, 128), :])
                nc.sync.dma_start(out=wq_t[:, i, :], in_=wqT_d[ts(i, 128), :])
            for i in range(8):
                nc.sync.dma_start(out=wv_t[:, i, :], in_=wvT_d[ts(i, 128), :])
            nc.sync.dma_start(out=m0p_t[:], in_=m0p_d[:])
            make_identity(nc, ident_t)
            nc.vector.memset(eps_t, LN_EPS)
            nc.vector.memset(zero64_t, 0.0)

            # ---- K-projection of block 0 (PE starts as soon as wk+kt0 land)
            ktproj0_t = kproj_pool.tile([128, 8, BLK], f32r, tag="ktproj_t")
            for o in range(8):
                ps = kps_pool.tile([128, 512], f32, tag="mm")
                for i in range(8):
                    nc.tensor.matmul(
                        ps,
                        wk_t[:, i, ts(o, 128)],
                        kt0_t[:, i, :],
                        start=(i == 0),
                        stop=(i == 7),
                    )
                nc.vector.tensor_copy(ktproj0_t[:, o, :], ps)

            # ---- Q projection: qproj[(h,d) chunk h][d, s] ----
            for o in range(8):
                qps = kps_pool.tile([128, 512], f32, tag="mm")
                for i in range(8):
                    nc.tensor.matmul(
                        qps[:, 0:S],
                        wq_t[:, i, ts(o, 128)],
                        qt_t[:, i, :],
                        start=(i == 0),
                        stop=(i == 7),
                    )
                nc.vector.tensor_copy(qproj_t[:, o, :], qps[:, 0:S])

            # ---- persistent AV accumulator: [d, h, s] (one PSUM bank) ----
            av_ps = avps_pool.tile([128, H, S], f32)

            for b in range(NBLK):
                if b == 0:
                    kt_t = kt0_t
                    biasF_t = biasF0_t
                    ktproj_t = ktproj0_t
                else:
                    # load k^T block
                    kt_t = ktp_pool.tile([128, 8, BLK], bf16, tag="kt_t")
                    for i in range(8):
                        nc.sync.dma_start(
                            out=kt_t[:, i, :], in_=kt_d[ts(i, 128), ts(b, BLK)]
                        )
                    biasF_t = bf_pool.tile([SPB * H, BLK], f32, tag="biasF_t")
                    nc.sync.dma_start(out=biasF_t, in_=biasF_d[b])

                    # K-projection (transposed out): ktp[(h,d) chunk][d, rows]
                    ktproj_t = kproj_pool.tile([128, 8, BLK], f32r, tag="ktproj_t")
                    for o in range(8):
                        ps = kps_pool.tile([128, 512], f32, tag="mm")
                        for i in range(8):
                            nc.tensor.matmul(
                                ps,
                                wk_t[:, i, ts(o, 128)],
                                kt_t[:, i, :],
                                start=(i == 0),
                                stop=(i == 7),
                            )
                        nc.vector.tensor_copy(ktproj_t[:, o, :], ps)

                # k block in normal layout (for attn@k aggregation)
                kn_t = kn_pool.tile([128, RC, DM], bf16)
                for rc in range(RC):
                    nc.sync.dma_start(
                        out=kn_t[:, rc, :],
                        in_=kn_d[b * BLK + rc * 128 : b * BLK + (rc + 1) * 128, :],
                    )

                # QK via block-diagonal Q: out[(s,h), rows]
                qk_ps = qkps_pool.tile([SPB * H, BLK], f32)
                for h in range(H):
                    bdq = bdq_pool.tile([128, SPB * H], f32r)
                    bdq_r = bdq.rearrange("p (s h) -> p s h", h=H)
                    nc.vector.tensor_copy(bdq, zero64_t)
                    nc.vector.tensor_copy(bdq_r[:, :, h], qproj_t[:, h, ts(b, SPB)])
                    nc.tensor.matmul(
                        qk_ps,
                        bdq,
                        ktproj_t[:, h, :],
                        start=(h == 0),
                        stop=(h == 7),
                    )

                # mask/bias over the FULL row (off-diagonal cols get -1e10,
                # so softmax over 512 cols == per-source softmax over 64)
                smf = sm_pool.tile([SPB * H, BLK], f32, tag="smf")
                nc.vector.tensor_add(smf, qk_ps, biasF_t)
                mx = st_pool.tile([SPB * H, 1], f32)
                nc.vector.reduce_max(mx, smf, axis=AX.X)
                negmx = st_pool.tile([SPB * H, 1], f32)
                nc.vector.tensor_scalar_mul(negmx, mx, -1.0)
                ssum = st_pool.tile([SPB * H, 1], f32)
                nc.scalar.activation(
                    smf, smf, AF.Exp, bias=negmx, scale=1.0, accum_out=ssum
                )
                rinv = st_pool.tile([SPB * H, 1], f32)
                nc.vector.reciprocal(rinv, ssum)
                nc.vector.tensor_scalar_mul(smf, smf, rinv)
                # attn output: 8 diagonal-block DMAs (DMA has no partition
                # alignment restriction)
                for sl in range(SPB):
                    nc.sync.dma_start(
                        out=attn_d[b * SPB * H + sl * H : b * SPB * H + (sl + 1) * H, :],
                        in_=smf[sl * H : (sl + 1) * H, ts(sl, NBR)],
                    )
                # transpose prob chunks -> [(s',n), (s,h)], mask to the
                # block-diagonal: BDall[p,(s,h)] = A[s,h,n(p)] * delta(s(p)==s)
                bdall = bdall_pool.tile([128, RC, SPB * H], bf16)
                for rc in range(RC):
                    pT = tpps_pool.tile([128, 512], f32, tag="tp")
                    nc.tensor.matmul(
                        pT[:, 0 : SPB * H],
                        smf[:, ts(rc, 128)],
                        ident_t[0 : SPB * H, 0 : SPB * H],
                        is_transpose=True,
                        start=True,
                        stop=True,
                    )
                    nc.vector.tensor_mul(
                        bdall[0:64, rc, :], m0p_t[0:64, rc, :], pT[0:64, 0 : SPB * H]
                    )
                    nc.vector.tensor_mul(
                        bdall[64:128, rc, :],
                        m0p_t[64:128, rc, :],
                        pT[64:128, 0 : SPB * H],
                    )

                # attn @ k aggregation: akT[i, (s,h)] += k[rows,i]^T BDall
                for i in range(8):
                    akps = vps_pool.tile([128, 512], f32, tag="vmm")
                    for rc in range(RC):
                        nc.tensor.matmul(
                            akps[:, 0 : SPB * H],
                            kn_t[:, rc, ts(i, 128)],
                            bdall[:, rc, :],
                            start=(rc == 0),
                            stop=(rc == RC - 1),
                        )
                    nc.vector.tensor_copy(
                        ak_sb[:, i, ts(b, SPB * H)], akps[:, 0 : SPB * H]
                    )

            # ---- second projection: attn_outT[(h,d), s] = WvT . akT ----
            for h in range(H):
                akr = ak_sb.rearrange("p i (s h2) -> p i s h2", h2=H)
                for i in range(8):
                    nc.tensor.matmul(
                        av_ps[:, h, :],
                        wv_t[:, i, ts(h, DK)],
                        akr[:, i, :, h],
                        start=(h == 0 and i == 0),
                        stop=(h == H - 1 and i == 7),
                    )

            # ---- tail consts (emitted late so startup DMA stays lean) ----
            for i in range(8):
                nc.sync.dma_start(out=fc_t[:, i, :], in_=fcT_d[ts(i, 128), :])
            for i in range(10):
                nc.sync.dma_start(out=fc1_t[:, i, :], in_=fc1T_d[ts(i, 128), :])
            for i in range(2):
                nc.sync.dma_start(out=fc2_t[:, i, :], in_=fc2T_d[ts(i, 128), :])
                nc.sync.dma_start(out=xcatT_t[:, 8 + i, :], in_=srcT_d[ts(i, 128), :])
            nc.sync.dma_start(out=qpfcb_t[:], in_=qpfcb_d[:])
            nc.sync.dma_start(out=fc1b_t[:], in_=fc1b_d[:])
            nc.sync.dma_start(out=fc2b_t[:], in_=fc2b_d[:])

            # ---- tail: fc + residual + LN + merge MLP ----
            av_sb = tail.tile([128, H, S], f32r)
            nc.vector.tensor_copy(av_sb, av_ps)

            x0 = tail.tile([S, DM], f32)
            for half in range(2):
                fps = kps_pool.tile([128, 512], f32, tag="mm")
                for hd in range(8):
                    nc.tensor.matmul(
                        fps[0:S, :],
                        av_sb[:, hd, :],
                        fc_t[:, hd, ts(half, 512)],
                        start=(hd == 0),
                        stop=(hd == 7),
                    )
                nc.vector.tensor_add(
                    x0[:, ts(half, 512)], fps[0:S, :], qpfcb_t[:, ts(half, 512)]
                )

            # LayerNorm over the 1024 free dim
            x0_r = x0.rearrange("p (a b) -> p a b", b=512)
            stats = tail.tile([S, 2, nc.vector.BN_STATS_DIM], f32)
            for sub in range(2):
                nc.vector.bn_stats(stats[:, sub, :], x0_r[:, sub, :])
            mv = tail.tile([S, nc.vector.BN_AGGR_DIM], f32)
            nc.vector.bn_aggr(mv, stats)
            sd = tail.tile([S, 1], f32)
            nc.scalar.activation(sd, mv[:, 1:2], AF.Sqrt, bias=eps_t, scale=1.0)
            rstd = tail.tile([S, 1], f32)
            nc.vector.reciprocal(rstd, sd)
            xn = x0
            nc.vector.tensor_scalar(
                out=xn,
                in0=x0,
                scalar1=mv[:, 0:1],
                scalar2=rstd,
                op0=mybir.AluOpType.subtract,
                op1=mybir.AluOpType.mult,
            )

            # transpose xn into xcatT chunks 0..7 (src already in 8..9)
            for c in range(8):
                tp = tpps_pool.tile([128, 512], f32, tag="tp")
                nc.tensor.transpose(tp[:, 0:S], xn[:, ts(c, 128)], ident_t[0:S, 0:S])
                nc.vector.tensor_copy(xcatT_t[:, c, :], tp[:, 0:S])

            # fc1 + relu
            h1ps_full = tpps_pool.tile([S, 512], f32, tag="tp")
            h1ps = h1ps_full[:, 0:FEAT]
            for c in range(10):
                nc.tensor.matmul(
                    h1ps,
                    xcatT_t[:, c, :],
                    fc1_t[:, c, :],
                    start=(c == 0),
                    stop=(c == 9),
                )
            h1 = tail.tile([S, FEAT], f32)
            nc.vector.tensor_add(h1, h1ps, fc1b_t)
            nc.scalar.activation(h1, h1, AF.Relu)

            # fc2
            h1T = tail.tile([128, 2, S], f32r)
            for c in range(2):
                tp = tpps_pool.tile([128, 512], f32, tag="tp")
                nc.tensor.transpose(tp[:, 0:S], h1[:, ts(c, 128)], ident_t[0:S, 0:S])
                nc.vector.tensor_copy(h1T[:, c, :], tp[:, 0:S])
            zps_full = tpps_pool.tile([S, 512], f32, tag="tp")
            zps = zps_full[:, 0:FEAT]
            for c in range(2):
                nc.tensor.matmul(
                    zps,
                    h1T[:, c, :],
                    fc2_t[:, c, :],
                    start=(c == 0),
                    stop=(c == 1),
                )
            z_sb = tail.tile([S, FEAT], f32)
            nc.vector.tensor_add(z_sb, zps, fc2b_t)
            nc.sync.dma_start(out=z_d[:], in_=z_sb)

    _fix_multiwait(nc)
    return nc


def _host_prep(inp):
    """Build per-core input dicts from full inputs (host numpy only)."""
    f = np.float32
    src = np.asarray(inp["src"], f)
    src_t = np.asarray(inp["src_t"], f)
    src_p = np.asarray(inp["src_p"], f)
    seq = np.asarray(inp["seq"], f)
    seq_t = np.asarray(inp["seq_t"], f)
    seq_e = np.asarray(inp["seq_e"], f)
    seq_p = np.asarray(inp["seq_p"], f)
    mask = np.asarray(inp["mask"])
    fc_b = np.asarray(inp["fc_b"], f)

    k = np.concatenate([seq, seq_e, seq_t, seq_p], axis=2)          # [B,NGH,DM]
    q = np.concatenate([src, np.zeros_like(src), src_t, src_p], axis=2)

    wqT = np.ascontiguousarray(np.asarray(inp["w_qs"], f).T.astype(ml_dtypes.bfloat16))
    wkT = np.ascontiguousarray(np.asarray(inp["w_ks"], f).T.astype(ml_dtypes.bfloat16))
    wvT = np.ascontiguousarray(np.asarray(inp["w_vs"], f).T.astype(ml_dtypes.bfloat16))
    fcT = np.ascontiguousarray(np.asarray(inp["fc_w"], f).T)
    ln_g = np.asarray(inp["ln_g"], f)
    ln_b = np.asarray(inp["ln_b"], f)
    fc1_w = np.asarray(inp["fc1_w"], f)
    # fold LayerNorm affine into fc1: h = fc1_w @ concat(xn*g+b, src) + b1
    fc1_w_mod = fc1_w.copy()
    fc1_w_mod[:, :DM] = fc1_w[:, :DM] * ln_g[None, :]
    fc1_b_mod = np.asarray(inp["fc1_b"], f) + fc1_w[:, :DM] @ ln_b
    fc1T = np.ascontiguousarray(fc1_w_mod.T)
    fc2T = np.ascontiguousarray(np.asarray(inp["fc2_w"], f).T)
    fc1b = np.ascontiguousarray(np.broadcast_to(fc1_b_mod, (S, FEAT)))
    fc2b = np.ascontiguousarray(np.broadcast_to(np.asarray(inp["fc2_b"], f), (S, FEAT)))

    m0p = np.zeros((128, RC, SPB * H), np.float32)
    for par in range(2):
        for rc in range(RC):
            sl = 2 * rc + par
            m0p[par * 64 : (par + 1) * 64, rc, sl * H : (sl + 1) * H] = 1.0

    scale = np.float32(DK ** -0.5)
    in_maps = []
    for core in range(NCORES):
        b0 = BPC * core
        k_c = k[b0 : b0 + BPC].reshape(ROWS, DM)
        q_c = q[b0 : b0 + BPC].reshape(S, DM)
        maskb = np.where(
            mask[b0 : b0 + BPC].reshape(S, NBR), f(-1e10), f(0.0)
        ).astype(f)
        bf = np.full((NBLK, SPB * H, BLK), f(-1e10), f)
        mb = maskb.reshape(NBLK, SPB, NBR)
        for sl in range(SPB):
            bf[:, sl * H : (sl + 1) * H, sl * NBR : (sl + 1) * NBR] = mb[
                :, sl, None, :
            ]
        in_maps.append(
            {
                "kt": np.ascontiguousarray(k_c.T.astype(ml_dtypes.bfloat16)),
                "qt": np.ascontiguousarray((q_c * scale).T.astype(ml_dtypes.bfloat16)),
                "qpfcb": np.ascontiguousarray(q_c + fc_b[None, :]),
                "srcT": np.ascontiguousarray(
                    src[b0 : b0 + BPC].reshape(S, FEAT).T
                ),
                "biasF": bf,
                "m0p": m0p,
                "kn": np.ascontiguousarray(k_c.astype(ml_dtypes.bfloat16)),
                "wqT": wqT,
                "wkT": wkT,
                "wvT": wvT,
                "fcT": fcT,
                "fc1T": fc1T,
                "fc2T": fc2T,
                "fc1b": fc1b,
                "fc2b": fc2b,
            }
        )
    return in_maps


def _install_trace_hook():
    """Register the NTFF profile hook (missing antenv.axon_hooks in image)."""
    import sys
    import types

    if "antenv.axon_hooks" in sys.modules:
        return
    import antenv

    mod = types.ModuleType("antenv.axon_hooks")
    _hook = [None]
    mod.set_axon_ntff_profile_hook = lambda h: _hook.__setitem__(0, h)
    mod.get_axon_ntff_profile_hook = lambda: _hook[0]
    sys.modules["antenv.axon_hooks"] = mod
    antenv.axon_hooks = mod
    try:
        from trn_agent_boot.trn_boot import _ntff_profile_via_ctypes

        h = _ntff_profile_via_ctypes("/opt/axon/libaxon_pjrt.so")
        if h is not None:
            mod.set_axon_ntff_profile_hook(h)
    except Exception:
        pass


def kernel(**inputs):
    global LAST_EXEC_NS
    from concourse.bass_utils import run_bass_kernel_spmd

    trace = bool(os.environ.get("BASS_KERNEL_TRACE"))
    if trace:
        _install_trace_hook()

    if "nc" not in _CACHE:
        _CACHE["nc"] = _build_nc()
    nc = _CACHE["nc"]

    in_maps = _host_prep(inputs)
    kwargs = {}
    if trace:
        kwargs["trace"] = True
        td = os.environ.get("BASS_KERNEL_TRACE_DIR")
        if td:
            os.makedirs(td, exist_ok=True)
            kwargs["tmpdir"] = td
    res = run_bass_kernel_spmd(nc, in_maps, list(range(NCORES)), **kwargs)
    LAST_EXEC_NS = res.exec_time_ns

    z = np.stack([res.results[i]["z"] for i in range(NCORES)]).reshape(B, NS, FEAT)
    attn = np.stack([res.results[i]["attn"] for i in range(NCORES)]).reshape(
        B, NS, H, NBR
    )
    return z, attn


# revision 41
# speedup vs baseline: 1.7899x; 1.0330x over previous
"""Trainium2 Bass kernel for the TGAT-style AttnModel (gnn_message_passing).

Contract: kernel(**inputs) takes FULL unsharded numpy inputs (as produced by
setup_inputs()) and returns the FULL output tuple (z, attn).

Strategy: pure data parallel over batch B=16 -> 2 batches per NeuronCore
(8 cores). Per core: 64 sources, 4096 neighbor rows. All projections run as
fp32r matmuls on the PE; attention uses a block-diagonal-Q trick for QK and a
block-diagonal-probs (BD) trick for attn@V; softmax in fp32 on DVE/ACT.
Host-side prep does the concats/transposes (pure data movement).
"""

import os
import numpy as np

import ml_dtypes

# ---- model constants (hardcoded; kernel.py must be self-contained) ----
B, NS, NGH, NBR = 16, 32, 2048, 64
FEAT = 256
DM, H, DK = 1024, 8, 128
LN_EPS = 1e-5

NCORES = 8
BPC = B // NCORES          # 2 batches per core
S = BPC * NS               # 64 sources per core
ROWS = BPC * NGH           # 4096 k-rows per core
NBLK = 8
BLK = ROWS // NBLK         # 512 rows per block
SPB = S // NBLK            # 8 sources per block
RC = BLK // 128            # 4 row-chunks of 128 per block

_CACHE = {}
LAST_EXEC_NS = None


def _patch_drain():
    """walrus in this container rejects instructions with >1 sync wait on the
    Drain ctrl struct; split the TileContext tail-drain waits into one drain
    per semaphore."""
    import concourse.tile as tile
    from concourse import mybir
    from concourse.vector_clock import ScopedClock

    if getattr(tile.TileContext, "_drain_patched", False):
        return

    def _drain_and_barrier(self, tick_clock, wait_clock):
        nc = self.nc
        drain_inst = nc.sync.drain()
        wait_clock.add_sem_waits(
            drain_inst.ins, ScopedClock({None: tick_clock.global_clock})
        )
        ri = drain_inst.ins
        waits = list(ri.sync_info.on_wait)
        ri.sync_info = mybir.SyncInfo(on_wait=waits[:1], on_update=[])
        for w in waits[1:]:
            d2 = nc.sync.drain()
            d2.ins.sync_info = mybir.SyncInfo(on_wait=[w], on_update=[])
        nc.all_engine_barrier()
        popped = nc._tile_sem_poison_stack.pop()
        assert popped is self._sem_poison
        nc.clear_and_free_semaphores(list(self.sems.allocated().values()))
        nc.all_engine_barrier()

    tile.TileContext._drain_and_barrier = _drain_and_barrier
    tile.TileContext._drain_patched = True


def _fix_multiwait(nc):
    """walrus in this container accepts very few sync commands per
    instruction (1 wait on NOP/Drain ctrl; a wait+update on engine ctrl).
    Conservatively rewrite every block so each instruction carries at most
    ONE wait: excess waits move to same-engine NoOp carriers inserted
    immediately before the instruction — identical semantics (same engine,
    same program position), so no scheduling or deadlock risk."""
    from concourse import mybir

    ctr = [0]
    for f in nc.m.functions:
        for bb in f.blocks:
            out = []
            changed = False
            for inst in bb.instructions:
                si = inst.sync_info
                if si is not None and len(si.on_wait) > 1:
                    waits = list(si.on_wait)
                    for w in waits[:-1]:
                        ctr[0] += 1
                        nop = mybir.InstDrain(
                            name=f"I-wfix-{ctr[0]}",
                            ins=[],
                            outs=[],
                            is_reset_sema=False,
                        )
                        nop.engine = inst.engine
                        nop.sync_info = mybir.SyncInfo(
                            on_wait=[w], on_update=[]
                        )
                        out.append(nop)
                    inst.sync_info = mybir.SyncInfo(
                        on_wait=[waits[-1]], on_update=list(si.on_update)
                    )
                    changed = True
                out.append(inst)
            if changed:
                bb.instructions = out


def _build_nc():
    import concourse.bass as bass
    import concourse.tile as tile
    import concourse.tile_utils as tile_utils
    from concourse import mybir
    from concourse.bass import ts
    from concourse.masks import make_identity

    _patch_drain()
    # stale 192KB cap; cayman has 208KB usable per partition
    tile_utils.max_sbuf_usage = 207 * 1024

    f32 = mybir.dt.float32
    f32r = mybir.dt.float32r
    bf16 = mybir.dt.bfloat16
    AX = mybir.AxisListType
    AF = mybir.ActivationFunctionType

    nc = bass.Bass(target_bir_lowering=False)

    # ---- DRAM parameters (per-core views, host-prepared) ----
    kt_d = nc.declare_dram_parameter("kt", [DM, ROWS], bf16, isOutput=False)
    qt_d = nc.declare_dram_parameter("qt", [DM, S], bf16, isOutput=False)
    qpfcb_d = nc.declare_dram_parameter("qpfcb", [S, DM], f32, isOutput=False)
    srcT_d = nc.declare_dram_parameter("srcT", [FEAT, S], f32r, isOutput=False)
    biasF_d = nc.declare_dram_parameter("biasF", [NBLK, SPB * H, BLK], f32, isOutput=False)
    kn_d = nc.declare_dram_parameter("kn", [ROWS, DM], bf16, isOutput=False)
    m0p_d = nc.declare_dram_parameter("m0p", [128, RC, SPB * H], f32, isOutput=False)
    wqT_d = nc.declare_dram_parameter("wqT", [DM, DM], bf16, isOutput=False)
    wkT_d = nc.declare_dram_parameter("wkT", [DM, DM], bf16, isOutput=False)
    wvT_d = nc.declare_dram_parameter("wvT", [DM, DM], bf16, isOutput=False)
    fcT_d = nc.declare_dram_parameter("fcT", [DM, DM], f32r, isOutput=False)
    fc1T_d = nc.declare_dram_parameter("fc1T", [DM + FEAT, FEAT], f32r, isOutput=False)
    fc2T_d = nc.declare_dram_parameter("fc2T", [FEAT, FEAT], f32r, isOutput=False)
    fc1b_d = nc.declare_dram_parameter("fc1b", [S, FEAT], f32, isOutput=False)
    fc2b_d = nc.declare_dram_parameter("fc2b", [S, FEAT], f32, isOutput=False)
    z_d = nc.declare_dram_parameter("z", [S, FEAT], f32, isOutput=True)
    attn_d = nc.declare_dram_parameter("attn", [S * H, NBR], f32, isOutput=True)

    with tile.TileContext(nc) as tc:
        with (
            tc.tile_pool(name="const", bufs=1) as const,
            tc.tile_pool(name="kt", bufs=2) as ktp_pool,
            tc.tile_pool(name="kproj", bufs=2) as kproj_pool,
            tc.tile_pool(name="kn", bufs=2) as kn_pool,
            tc.tile_pool(name="bdq", bufs=2) as bdq_pool,
            tc.tile_pool(name="sm", bufs=2) as sm_pool,
            tc.tile_pool(name="bf", bufs=2) as bf_pool,
            tc.tile_pool(name="st", bufs=8) as st_pool,
            tc.tile_pool(name="bdall", bufs=2) as bdall_pool,
            tc.tile_pool(name="tail", bufs=1) as tail,
            tc.tile_pool(name="kps", bufs=2, space="PSUM") as kps_pool,
            tc.tile_pool(name="vps", bufs=2, space="PSUM") as vps_pool,
            tc.tile_pool(name="qkps", bufs=1, space="PSUM") as qkps_pool,
            tc.tile_pool(name="avps", bufs=1, space="PSUM") as avps_pool,
            tc.tile_pool(name="tpps", bufs=2, space="PSUM") as tpps_pool,
        ):
            # ---- constants / weights ----
            wk_t = const.tile([128, 8, DM], bf16)
            wv_t = const.tile([128, 8, DM], bf16)
            fc_t = const.tile([128, 8, DM], f32r)  # holds wqT first, fcT later
            fc1_t = const.tile([128, 10, FEAT], f32r)
            fc2_t = const.tile([128, 2, FEAT], f32r)
            m0p_t = const.tile([128, RC, SPB * H], f32)
            ak_sb = const.tile([128, 8, S * H], bf16)
            qpfcb_t = const.tile([S, DM], f32)
            fc1b_t = const.tile([S, FEAT], f32)
            fc2b_t = const.tile([S, FEAT], f32)
            xcatT_t = const.tile([128, 10, S], f32r)
            qproj_t = const.tile([128, 8, S], f32)
            ident_t = const.tile([128, 128], f32)
            eps_t = const.tile([S, 1], f32)
            zero64_t = const.tile([128, SPB * H], f32)

            # startup-ordered loads: wk -> kt(b0)+biasF(b0) -> qt+wqT -> wv -> m0
            for i in range(8):
                nc.sync.dma_start(out=wk_t[:, i, :], in_=wkT_d[ts(i, 128), :])
            kt0_t = ktp_pool.tile([128, 8, BLK], bf16, tag="kt_t")
            for i in range(8):
                nc.sync.dma_start(out=kt0_t[:, i, :], in_=kt_d[ts(i, 128), ts(0, BLK)])
            biasF0_t = bf_pool.tile([SPB * H, BLK], f32, tag="biasF_t")
            nc.sync.dma_start(out=biasF0_t, in_=biasF_d[0])
            qt_t = const.tile([128, 8, S], bf16)
            wq_t = const.tile([128, 8, DM], bf16)
            for i in range(8):
                nc.sync.dma_start(out=qt_t[:, i, :], in_=qt_d[ts(i, 128), :])
                nc.sync.dma_start(out=wq_t[:, i, :], in_=wqT_d[ts(i, 128), :])
            nc.sync.dma_start(out=m0p_t[:], in_=m0p_d[:])
            make_identity(nc, ident_t)
            nc.vector.memset(eps_t, LN_EPS)
            nc.vector.memset(zero64_t, 0.0)

            # ---- K-projection of block 0 (PE starts as soon as wk+kt0 land)
            ktproj0_t = kproj_pool.tile([128, 8, BLK], f32r, tag="ktproj_t")
            for o in range(8):
                ps = kps_pool.tile([128, 512], f32, tag="mm")
                for i in range(8):
                    nc.tensor.matmul(
                        ps,
                        wk_t[:, i, ts(o, 128)],
                        kt0_t[:, i, :],
                        start=(i == 0),
                        stop=(i == 7),
                    )
                nc.vector.tensor_copy(ktproj0_t[:, o, :], ps)

            # ---- Q projection: qproj[(h,d) chunk h][d, s] ----
            for o in range(8):
                qps = kps_pool.tile([128, 512], f32, tag="mm")
                for i in range(8):
                    nc.tensor.matmul(
                        qps[:, 0:S],
                        wq_t[:, i, ts(o, 128)],
                        qt_t[:, i, :],
                        start=(i == 0),
                        stop=(i == 7),
                    )
                nc.vector.tensor_copy(qproj_t[:, o, :], qps[:, 0:S])

            # ---- persistent AV accumulator: [d, h, s] (one PSUM bank) ----
            av_ps = avps_pool.tile([128, H, S], f32)

            for b in range(NBLK):
                if b == 0:
                    kt_t = kt0_t
                    biasF_t = biasF0_t
                    ktproj_t = ktproj0_t
                else:
                    # load k^T block
                    kt_t = ktp_pool.tile([128, 8, BLK], bf16, tag="kt_t")
                    for i in range(8):
                        nc.sync.dma_start(
                            out=kt_t[:, i, :], in_=kt_d[ts(i, 128), ts(b, BLK)]
                        )
                    biasF_t = bf_pool.tile([SPB * H, BLK], f32, tag="biasF_t")
                    nc.sync.dma_start(out=biasF_t, in_=biasF_d[b])

                    # K-projection (transposed out): ktp[(h,d) chunk][d, rows]
                    ktproj_t = kproj_pool.tile([128, 8, BLK], f32r, tag="ktproj_t")
                    for o in range(8):
                        ps = kps_pool.tile([128, 512], f32, tag="mm")
                        for i in range(8):
                            nc.tensor.matmul(
                                ps,
                                wk_t[:, i, ts(o, 128)],
                                kt_t[:, i, :],
                                start=(i == 0),
                                stop=(i == 7),
                            )
                        nc.vector.tensor_copy(ktproj_t[:, o, :], ps)

                # k block in normal layout (for attn@k aggregation)
                kn_t = kn_pool.tile([128, RC, DM], bf16)
                for rc in range(RC):
                    nc.sync.dma_start(
                        out=kn_t[:, rc, :],
                        in_=kn_d[b * BLK + rc * 128 : b * BLK + (rc + 1) * 128, :],
                    )

                # QK via block-diagonal Q: out[(s,h), rows]
                qk_ps = qkps_pool.tile([SPB * H, BLK], f32)
                for h in range(H):
                    bdq = bdq_pool.tile([128, SPB * H], f32r)
                    bdq_r = bdq.rearrange("p (s h) -> p s h", h=H)
                    nc.vector.tensor_copy(bdq, zero64_t)
                    nc.vector.tensor_copy(bdq_r[:, :, h], qproj_t[:, h, ts(b, SPB)])
                    nc.tensor.matmul(
                        qk_ps,
                        bdq,
                        ktproj_t[:, h, :],
                        start=(h == 0),
                        stop=(h == 7),
                    )

                # mask/bias over the FULL row (off-diagonal cols get -1e10,
                # so softmax over 512 cols == per-source softmax over 64)
                smf = sm_pool.tile([SPB * H, BLK], f32, tag="smf")
                nc.vector.tensor_add(smf, qk_ps, biasF_t)
                mx = st_pool.tile([SPB * H, 1], f32)
                nc.vector.reduce_max(mx, smf, axis=AX.X)
                negmx = st_pool.tile([SPB * H, 1], f32)
                nc.vector.tensor_scalar_mul(negmx, mx, -1.0)
                ssum = st_pool.tile([SPB * H, 1], f32)
                nc.scalar.activation(
                    smf, smf, AF.Exp, bias=negmx, scale=1.0, accum_out=ssum
                )
                rinv = st_pool.tile([SPB * H, 1], f32)
                nc.vector.reciprocal(rinv, ssum)
                nc.vector.tensor_scalar_mul(smf, smf, rinv)
                # attn output: 8 diagonal-block DMAs (DMA has no partition
                # alignment restriction)
                for sl in range(SPB):
                    nc.sync.dma_start(
                        out=attn_d[b * SPB * H + sl * H : b * SPB * H + (sl + 1) * H, :],
                        in_=smf[sl * H : (sl + 1) * H, ts(sl, NBR)],
                    )
                # transpose prob chunks -> [(s',n), (s,h)], mask to the
                # block-diagonal: BDall[p,(s,h)] = A[s,h,n(p)] * delta(s(p)==s)
                bdall = bdall_pool.tile([128, RC, SPB * H], bf16)
                for rc in range(RC):
                    pT = tpps_pool.tile([128, 512], f32, tag="tp")
                    nc.tensor.matmul(
                        pT[:, 0 : SPB * H],
                        smf[:, ts(rc, 128)],
                        ident_t[0 : SPB * H, 0 : SPB * H],
                        is_transpose=True,
                        start=True,
                        stop=True,
                    )
                    nc.vector.tensor_mul(
                        bdall[0:64, rc, :], m0p_t[0:64, rc, :], pT[0:64, 0 : SPB * H]
                    )
                    nc.vector.tensor_mul(
                        bdall[64:128, rc, :],
                        m0p_t[64:128, rc, :],
                        pT[64:128, 0 : SPB * H],
                    )

                # attn @ k aggregation: akT[i, (s,h)] += k[rows,i]^T BDall
                for i in range(8):
                    akps = vps_pool.tile([128, 512], f32, tag="vmm")
                    for rc in range(RC):
                        nc.tensor.matmul(
                            akps[:, 0 : SPB * H],
                            kn_t[:, rc, ts(i, 128)],
                            bdall[:, rc, :],
                            start=(rc == 0),
                            stop=(rc == RC - 1),
                        )
                    nc.vector.tensor_copy(
                        ak_sb[:, i, ts(b, SPB * H)], akps[:, 0 : SPB * H]
                    )

            # wv is only needed here; emitted late to keep startup DMA lean
            for i in range(8):
                nc.sync.dma_start(out=wv_t[:, i, :], in_=wvT_d[ts(i, 128), :])

            # ---- second projection: attn_outT[(h,d), s] = WvT . akT ----
            for h in range(H):
                akr = ak_sb.rearrange("p i (s h2) -> p i s h2", h2=H)
                for i in range(8):
                    nc.tensor.matmul(
                        av_ps[:, h, :],
                        wv_t[:, i, ts(h, DK)],
                        akr[:, i, :, h],
                        start=(h == 0 and i == 0),
                        stop=(h == H - 1 and i == 7),
                    )

            # ---- tail consts (emitted late so startup DMA stays lean) ----
            for i in range(8):
                nc.sync.dma_start(out=fc_t[:, i, :], in_=fcT_d[ts(i, 128), :])
            for i in range(10):
                nc.sync.dma_start(out=fc1_t[:, i, :], in_=fc1T_d[ts(i, 128), :])
            for i in range(2):
                nc.sync.dma_start(out=fc2_t[:, i, :], in_=fc2T_d[ts(i, 128), :])
                nc.sync.dma_start(out=xcatT_t[:, 8 + i, :], in_=srcT_d[ts(i, 128), :])
            nc.sync.dma_start(out=qpfcb_t[:], in_=qpfcb_d[:])
            nc.sync.dma_start(out=fc1b_t[:], in_=fc1b_d[:])
            nc.sync.dma_start(out=fc2b_t[:], in_=fc2b_d[:])

            # ---- tail: fc + residual + LN + merge MLP ----
            av_sb = tail.tile([128, H, S], f32r)
            nc.vector.tensor_copy(av_sb, av_ps)

            x0 = tail.tile([S, DM], f32)
            for half in range(2):
                fps = kps_pool.tile([128, 512], f32, tag="mm")
                for hd in range(8):
                    nc.tensor.matmul(
                        fps[0:S, :],
                        av_sb[:, hd, :],
                        fc_t[:, hd, ts(half, 512)],
                        start=(hd == 0),
                        stop=(hd == 7),
                    )
                nc.vector.tensor_add(
                    x0[:, ts(half, 512)], fps[0:S, :], qpfcb_t[:, ts(half, 512)]
                )

            # LayerNorm over the 1024 free dim
            x0_r = x0.rearrange("p (a b) -> p a b", b=512)
            stats = tail.tile([S, 2, nc.vector.BN_STATS_DIM], f32)
            for sub in range(2):
                nc.vector.bn_stats(stats[:, sub, :], x0_r[:, sub, :])
            mv = tail.tile([S, nc.vector.BN_AGGR_DIM], f32)
            nc.vector.bn_aggr(mv, stats)
            sd = tail.tile([S, 1], f32)
            nc.scalar.activation(sd, mv[:, 1:2], AF.Sqrt, bias=eps_t, scale=1.0)
            rstd = tail.tile([S, 1], f32)
            nc.vector.reciprocal(rstd, sd)
            xn = x0
            nc.vector.tensor_scalar(
                out=xn,
                in0=x0,
                scalar1=mv[:, 0:1],
                scalar2=rstd,
                op0=mybir.AluOpType.subtract,
                op1=mybir.AluOpType.mult,
            )

            # transpose xn into xcatT chunks 0..7 (src already in 8..9)
            for c in range(8):
                tp = tpps_pool.tile([128, 512], f32, tag="tp")
                nc.tensor.transpose(tp[:, 0:S], xn[:, ts(c, 128)], ident_t[0:S, 0:S])
                nc.vector.tensor_copy(xcatT_t[:, c, :], tp[:, 0:S])

            # fc1 + relu
            h1ps_full = tpps_pool.tile([S, 512], f32, tag="tp")
            h1ps = h1ps_full[:, 0:FEAT]
            for c in range(10):
                nc.tensor.matmul(
                    h1ps,
                    xcatT_t[:, c, :],
                    fc1_t[:, c, :],
                    start=(c == 0),
                    stop=(c == 9),
                )
            h1 = tail.tile([S, FEAT], f32)
            nc.vector.tensor_add(h1, h1ps, fc1b_t)
            nc.scalar.activation(h1, h1, AF.Relu)

            # fc2
            h1T = tail.tile([128, 2, S], f32r)
            for c in range(2):
                tp = tpps_pool.tile([128, 512], f32, tag="tp")
                nc.tensor.transpose(tp[:, 0:S], h1[:, ts(c, 128)], ident_t[0:S, 0:S])
                nc.vector.tensor_copy(h1T[:, c, :], tp[:, 0:S])
            zps_full = tpps_pool.tile([S, 512], f32, tag="tp")
            zps = zps_full[:, 0:FEAT]
            for c in range(2):
                nc.tensor.matmul(
                    zps,
                    h1T[:, c, :],
                    fc2_t[:, c, :],
                    start=(c == 0),
                    stop=(c == 1),
                )
            z_sb = tail.tile([S, FEAT], f32)
            nc.vector.tensor_add(z_sb, zps, fc2b_t)
            nc.sync.dma_start(out=z_d[:], in_=z_sb)

    _fix_multiwait(nc)
    return nc


def _host_prep(inp):
    """Build per-core input dicts from full inputs (host numpy only)."""
    f = np.float32
    src = np.asarray(inp["src"], f)
    src_t = np.asarray(inp["src_t"], f)
    src_p = np.asarray(inp["src_p"], f)
    seq = np.asarray(inp["seq"], f)
    seq_t = np.asarray(inp["seq_t"], f)
    seq_e = np.asarray(inp["seq_e"], f)
    seq_p = np.asarray(inp["seq_p"], f)
    mask = np.asarray(inp["mask"])
    fc_b = np.asarray(inp["fc_b"], f)

    k = np.concatenate([seq, seq_e, seq_t, seq_p], axis=2)          # [B,NGH,DM]
    q = np.concatenate([src, np.zeros_like(src), src_t, src_p], axis=2)

    wqT = np.ascontiguousarray(np.asarray(inp["w_qs"], f).T.astype(ml_dtypes.bfloat16))
    wkT = np.ascontiguousarray(np.asarray(inp["w_ks"], f).T.astype(ml_dtypes.bfloat16))
    wvT = np.ascontiguousarray(np.asarray(inp["w_vs"], f).T.astype(ml_dtypes.bfloat16))
    fcT = np.ascontiguousarray(np.asarray(inp["fc_w"], f).T)
    ln_g = np.asarray(inp["ln_g"], f)
    ln_b = np.asarray(inp["ln_b"], f)
    fc1_w = np.asarray(inp["fc1_w"], f)
    # fold LayerNorm affine into fc1: h = fc1_w @ concat(xn*g+b, src) + b1
    fc1_w_mod = fc1_w.copy()
    fc1_w_mod[:, :DM] = fc1_w[:, :DM] * ln_g[None, :]
    fc1_b_mod = np.asarray(inp["fc1_b"], f) + fc1_w[:, :DM] @ ln_b
    fc1T = np.ascontiguousarray(fc1_w_mod.T)
    fc2T = np.ascontiguousarray(np.asarray(inp["fc2_w"], f).T)
    fc1b = np.ascontiguousarray(np.broadcast_to(fc1_b_mod, (S, FEAT)))
    fc2b = np.ascontiguousarray(np.broadcast_to(np.asarray(inp["fc2_b"], f), (S, FEAT)))

    m0p = np.zeros((128, RC, SPB * H), np.float32)
    for par in range(2):
        for rc in range(RC):
            sl = 2 * rc + par
            m0p[par * 64 : (par + 1) * 64, rc, sl * H : (sl + 1) * H] = 1.0

    scale = np.float32(DK ** -0.5)
    in_maps = []
    for core in range(NCORES):
        b0 = BPC * core
        k_c = k[b0 : b0 + BPC].reshape(ROWS, DM)
        q_c = q[b0 : b0 + BPC].reshape(S, DM)
        maskb = np.where(
            mask[b0 : b0 + BPC].reshape(S, NBR), f(-1e10), f(0.0)
        ).astype(f)
        bf = np.full((NBLK, SPB * H, BLK), f(-1e10), f)
        mb = maskb.reshape(NBLK, SPB, NBR)
        for sl in range(SPB):
            bf[:, sl * H : (sl + 1) * H, sl * NBR : (sl + 1) * NBR] = mb[
                :, sl, None, :
            ]
        in_maps.append(
            {
                "kt": np.ascontiguousarray(k_c.T.astype(ml_dtypes.bfloat16)),
                "qt": np.ascontiguousarray((q_c * scale).T.astype(ml_dtypes.bfloat16)),
                "qpfcb": np.ascontiguousarray(q_c + fc_b[None, :]),
                "srcT": np.ascontiguousarray(
                    src[b0 : b0 + BPC].reshape(S, FEAT).T
                ),
                "biasF": bf,
                "m0p": m0p,
                "kn": np.ascontiguousarray(k_c.astype(ml_dtypes.bfloat16)),
                "wqT": wqT,
                "wkT": wkT,
                "wvT": wvT,
                "fcT": fcT,
                "fc1T": fc1T,
                "fc2T": fc2T,
                "fc1b": fc1b,
                "fc2b": fc2b,
            }
        )
    return in_maps


def _install_trace_hook():
    """Register the NTFF profile hook (missing antenv.axon_hooks in image)."""
    import sys
    import types

    if "antenv.axon_hooks" in sys.modules:
        return
    import antenv

    mod = types.ModuleType("antenv.axon_hooks")
    _hook = [None]
    mod.set_axon_ntff_profile_hook = lambda h: _hook.__setitem__(0, h)
    mod.get_axon_ntff_profile_hook = lambda: _hook[0]
    sys.modules["antenv.axon_hooks"] = mod
    antenv.axon_hooks = mod
    try:
        from trn_agent_boot.trn_boot import _ntff_profile_via_ctypes

        h = _ntff_profile_via_ctypes("/opt/axon/libaxon_pjrt.so")
        if h is not None:
            mod.set_axon_ntff_profile_hook(h)
    except Exception:
        pass


def kernel(**inputs):
    global LAST_EXEC_NS
    from concourse.bass_utils import run_bass_kernel_spmd

    trace = bool(os.environ.get("BASS_KERNEL_TRACE"))
    if trace:
        _install_trace_hook()

    if "nc" not in _CACHE:
        _CACHE["nc"] = _build_nc()
    nc = _CACHE["nc"]

    in_maps = _host_prep(inputs)
    kwargs = {}
    if trace:
        kwargs["trace"] = True
        td = os.environ.get("BASS_KERNEL_TRACE_DIR")
        if td:
            os.makedirs(td, exist_ok=True)
            kwargs["tmpdir"] = td
    res = run_bass_kernel_spmd(nc, in_maps, list(range(NCORES)), **kwargs)
    LAST_EXEC_NS = res.exec_time_ns

    z = np.stack([res.results[i]["z"] for i in range(NCORES)]).reshape(B, NS, FEAT)
    attn = np.stack([res.results[i]["attn"] for i in range(NCORES)]).reshape(
        B, NS, H, NBR
    )
    return z, attn
